# revision 1
# baseline (speedup 1.0000x reference)
"""Trainium2 Bass kernel for a 2-layer GCN encoder (GCNConv -> LN -> GELU -> GCNConv -> LN).

Strategy (8 NeuronCores, SPMD):
  - Nodes are assigned to 8 cores x TPC tiles of 128 dst-slots each, balanced by
    in-degree so every tile aggregates ~the same number of edges.
  - Per layer: transform features locally (X @ W on the node shard), AllGather the
    transformed table to every core's HBM, then each core aggregates its dst tiles:
    dma_gather of source rows (int16 indices against 4 table chunks), segment-sum
    via one-hot selector matmuls accumulating in PSUM, then bias + LayerNorm (+GELU).
  - Self-loops are folded in as ordinary edges with coeff 1/deg.
"""

from contextlib import ExitStack

import numpy as np

import concourse.bass as bass
import concourse.bacc as bacc
import concourse.mybir as mybir
import concourse.tile as tile
from concourse.bass_utils import run_bass_kernel_spmd

dt = mybir.dt
F32 = dt.float32
BF16 = dt.bfloat16

# -------- problem geometry (hardcoded for the graded problem) --------
N_FULL = 100000
IN_DIM = 256
HID2 = 256
HID = 128
N_CORES = 8
TILE = 128
TPC = 98          # tiles per core -> shard = 12544 >= 12500
NCHUNK = 4        # int16 gather index range / table chunking
GMAX = 8          # max blocks (x128 idxs) per dma_gather call (HW limit 1024 idxs)


# ============================ host preprocessing ============================

def preprocess(x, edge_index, n_cores, tpc):
    """Balanced node->tile assignment + per-core edge arrays."""
    N = x.shape[0]
    in_dim = x.shape[1]
    shard = tpc * TILE
    padn = n_cores * shard
    ch = padn // NCHUNK
    assert ch <= 32768 and padn % NCHUNK == 0

    src = np.asarray(edge_index[0], np.int64)
    dst = np.asarray(edge_index[1], np.int64)

    deg = (np.bincount(dst, minlength=N) + 1).astype(np.float32)
    dinv = (1.0 / np.sqrt(deg)).astype(np.float32)

    # --- balanced assignment: stride the degree-sorted nodes across tiles ---
    NT = n_cores * tpc
    assert N <= NT * TILE
    order = np.argsort(-deg, kind="stable")
    node_tile = np.empty(N, np.int32)
    node_slot = np.empty(N, np.int32)
    ar = np.arange(N, dtype=np.int64)
    node_tile[order] = (ar % NT).astype(np.int32)
    node_slot[order] = (ar // NT).astype(np.int32)
    core_of = node_tile % n_cores
    lt_of = node_tile // n_cores
    row_of = core_of.astype(np.int64) * shard + lt_of.astype(np.int64) * TILE + node_slot

    # --- edge arrays (self loops appended), grouped by (dst tile, src chunk) ---
    a_srcrow = np.concatenate([row_of[src], row_of])
    a_co = np.concatenate([(dinv[src] * dinv[dst]).astype(np.float32),
                           (dinv * dinv).astype(np.float32)])
    a_dtile = np.concatenate([node_tile[dst], node_tile]).astype(np.int64)
    a_dslot = np.concatenate([node_slot[dst], node_slot]).astype(np.float32)
    a_chunk = a_srcrow // ch

    key = a_dtile * NCHUNK + a_chunk
    o2 = np.argsort(key, kind="stable")
    s_srcrow = a_srcrow[o2]
    s_co = a_co[o2]
    s_dslot = a_dslot[o2]

    cnts = np.bincount(key, minlength=NT * NCHUNK)
    starts = np.zeros(NT * NCHUNK + 1, np.int64)
    np.cumsum(cnts, out=starts[1:])
    # tile id t = lt * n_cores + core  -> counts[lt, core, chunk]
    cnts_lkc = cnts.reshape(tpc, n_cores, NCHUNK)
    B = np.maximum(1, -(-cnts_lkc.max(axis=1) // TILE)).astype(np.int64)  # [tpc, NCHUNK]
    # blocks guaranteed fully written on every core (memset only above this)
    Bmin = np.minimum(B, np.maximum(cnts_lkc.min(axis=1), 1) // TILE).astype(np.int64)
    blk_off = np.zeros((tpc, NCHUNK), np.int64)
    run = 0
    for lt in range(tpc):
        for c in range(NCHUNK):
            blk_off[lt, c] = run
            run += int(B[lt, c])
    NB = int(run)

    n_subcalls = int(sum(-(-int(B[lt, c]) // GMAX)
                         for lt in range(tpc) for c in range(NCHUNK)))
    per_core = []
    for k in range(n_cores):
        idx_a = np.full((128, NB * 8), -1, np.int16)
        co_a = np.zeros((128, NB), np.float32)
        dl_a = np.zeros((128, NB), np.float32)
        cnt_a = np.zeros(n_subcalls, np.int32)
        sc = 0
        for lt in range(tpc):
            for c in range(NCHUNK):
                t = lt * n_cores + k
                m = int(cnts[t * NCHUNK + c])
                boff = int(blk_off[lt, c])
                bc = int(B[lt, c])
                if m > 0:
                    s0 = int(starts[t * NCHUNK + c])
                    sl = slice(s0, s0 + m)
                    j = np.arange(m)
                    co_a[j % 128, boff + j // 128] = s_co[sl]
                    dl_a[j % 128, boff + j // 128] = s_dslot[sl]
                    idx_a[j % 16, boff * 8 + j // 16] = \
                        (s_srcrow[sl] - c * ch).astype(np.int16)
                for q in range(0, bc, GMAX):
                    mv = min(max(m - q * TILE, 0), min(GMAX, bc - q) * TILE)
                    if mv == 0:
                        # >=1 valid index per call (all-negative breaks the DGE)
                        idx_a[0, (boff + q) * 8] = 0
                        mv = 1
                    cnt_a[sc] = mv
                    sc += 1
        assert sc == n_subcalls
        idx_a[16:, :] = np.tile(idx_a[:16, :], (7, 1))

        mask = core_of == k
        nodes_k = np.nonzero(mask)[0]
        pos_k = lt_of[nodes_k] * TILE + node_slot[nodes_k]
        xs = np.zeros((shard, in_dim), np.float32)
        xs[pos_k] = np.asarray(x, np.float32)[nodes_k]
        per_core.append(dict(xt=np.ascontiguousarray(xs.T), idx=idx_a, co=co_a, dl=dl_a,
                             cnt=cnt_a.reshape(1, -1), nodes=nodes_k, pos=pos_k))

    geom = dict(n_cores=n_cores, tpc=tpc, shard=shard, padn=padn, ch=ch,
                B=B, Bmin=Bmin, blk_off=blk_off, NB=NB, in_dim=in_dim,
                n_subcalls=n_subcalls)
    return geom, per_core


# ============================ bass program builder ============================

def build_program(tc, io, geom, tab1_dt=F32, sel1_dt=F32):
    nc = tc.nc
    tpc = geom["tpc"]
    shard = geom["shard"]
    padn = geom["padn"]
    ch = geom["ch"]
    B = geom["B"]
    blk_off = geom["blk_off"]
    NB = geom["NB"]
    in_dim = geom["in_dim"]
    n_in_ch = in_dim // 128
    n_h_ch = HID2 // 128
    HGRP = [(0, NCHUNK // 2), (NCHUNK // 2, NCHUNK)]
    BH_MAX = max(int(B[lt, lo:hi].sum()) for lt in range(tpc) for (lo, hi) in HGRP)
    eps = 1e-5
    AOT = mybir.AluOpType
    AFT = mybir.ActivationFunctionType
    mixed_sel = sel1_dt != F32

    ctx = ExitStack()
    consts = ctx.enter_context(tc.tile_pool(name="consts", bufs=1))
    work = ctx.enter_context(tc.tile_pool(name="work", bufs=2))
    ln = ctx.enter_context(tc.tile_pool(name="ln", bufs=3))
    msgp = ctx.enter_context(tc.tile_pool(name="msgp", bufs=2))
    selp = ctx.enter_context(tc.tile_pool(name="selp", bufs=2))
    ps256 = ctx.enter_context(tc.tile_pool(name="ps256", bufs=3, space="PSUM"))
    ps128 = ctx.enter_context(tc.tile_pool(name="ps128", bufs=2, space="PSUM"))
    dram = ctx.enter_context(tc.tile_pool(name="dram", bufs=1, space="DRAM"))

    # ---- constants into SBUF ----
    w1s = consts.tile([128, n_in_ch, HID2], F32)
    nc.sync.dma_start(w1s[:], io["w1"].rearrange("(c p) n -> p c n", p=128))
    w2s = consts.tile([128, n_h_ch, HID], F32)
    nc.sync.dma_start(w2s[:], io["w2"].rearrange("(c p) n -> p c n", p=128))
    bias1 = consts.tile([128, 3, HID2], F32)
    nc.sync.dma_start(bias1[:], io["bias1"])
    bias2 = consts.tile([128, 3, HID], F32)
    nc.sync.dma_start(bias2[:], io["bias2"])
    ident = consts.tile([128, 128], F32)
    nc.sync.dma_start(ident[:], io["ident"])
    idx_s = consts.tile([128, NB * 8], dt.int16)
    nc.sync.dma_start(idx_s[:], io["idx"])
    iota32 = consts.tile([128, 128], F32)
    nc.sync.dma_start(iota32[:], io["iota32"])
    co32 = consts.tile([128, NB], F32)
    nc.sync.dma_start(co32[:], io["co32"])
    dl32 = consts.tile([128, NB], F32)
    nc.sync.dma_start(dl32[:], io["dl32"])
    if mixed_sel:
        iota_l1 = consts.tile([128, 128], sel1_dt)
        nc.sync.dma_start(iota_l1[:], io["iota_b"])
        co_l1 = consts.tile([128, NB], sel1_dt)
        nc.sync.dma_start(co_l1[:], io["co_b"])
        dl_l1 = consts.tile([128, NB], sel1_dt)
        nc.sync.dma_start(dl_l1[:], io["dl_b"])
    else:
        iota_l1, co_l1, dl_l1 = iota32, co32, dl32

    # ---- DRAM collective buffers ----
    ag1_in = dram.tile([shard, HID2], tab1_dt)
    ag1_out = dram.tile([padn, HID2], tab1_dt, addr_space="Shared")
    ag2_in = dram.tile([shard, HID], F32)
    ag2_out = dram.tile([padn, HID], F32, addr_space="Shared")

    eps_t = consts.tile([128, 1], F32)
    nc.vector.memset(eps_t[:], eps)

    n_subcalls = geom["n_subcalls"]
    cnt_s = consts.tile([1, n_subcalls], dt.int32)
    nc.sync.dma_start(cnt_s[:], io["cnt"])
    cnt_regs = [nc.alloc_register(mybir.EngineType.Pool, f"gcnt{i}")
                for i in range(8)]
    sc_of = {}
    _sc = 0
    for _lt in range(tpc):
        for _c in range(NCHUNK):
            for _q in range(0, int(B[_lt, _c]), GMAX):
                sc_of[(_lt, _c, _q)] = _sc
                _sc += 1
    assert _sc == n_subcalls

    # ---- stage A: H1 = X @ W1 (shard-local) ----
    for lt in range(tpc):
        xt_t = work.tile([128, n_in_ch, 128], F32, tag="xt")
        nc.sync.dma_start(
            xt_t[:],
            io["xt"][:, lt * 128:(lt + 1) * 128].rearrange("(c p) n -> p c n", p=128))
        ps = ps256.tile([128, HID2], F32, tag="psAgg")
        for c in range(n_in_ch):
            nc.tensor.matmul(ps[:], xt_t[:, c, :], w1s[:, c, :],
                             start=(c == 0), stop=(c == n_in_ch - 1))
        h1t = work.tile([128, HID2], tab1_dt, tag="h1t")
        nc.vector.tensor_copy(h1t[:], ps[:])
        nc.sync.dma_start(ag1_in[lt * 128:(lt + 1) * 128, :], h1t[:])

    nc.gpsimd.collective_compute(
        "AllGather", AOT.bypass,
        replica_groups=[list(range(geom["n_cores"]))],
        ins=[ag1_in.opt()], outs=[ag1_out.opt()])

    # ---- generic aggregation + LN (+ gelu) ----
    def agg_layer(tab_ap, feat, sel_dtype, co_t, dl_t, iota_t, bias_t, gelu, out_cb):
        for lt in range(tpc):
            bt_total = int(B[lt].sum())
            ps = ps256.tile([128, feat], F32, tag="psAgg")
            done = 0
            for (lo, hi) in HGRP:
                bh = int(B[lt, lo:hi].sum())
                if bh == 0:
                    continue
                boff = int(blk_off[lt, lo])
                msg = msgp.tile([128, BH_MAX, feat], sel_dtype, tag="msg")
                for c in range(lo, hi):
                    bc = int(B[lt, c])
                    if bc == 0:
                        continue
                    moff = int(blk_off[lt, c]) - boff
                    bmin = int(geom["Bmin"][lt, c])
                    if bmin < bc:
                        # slots >= per-core count are skipped by the gather;
                        # zero them so the selector matmul never sees NaN bits
                        nc.vector.memset(msg[:, moff + bmin:moff + bc, :], 0.0)
                    for q in range(0, bc, GMAX):
                        bq = min(GMAX, bc - q)
                        sc = sc_of[(lt, c, q)]
                        reg = cnt_regs[sc % len(cnt_regs)]
                        nc.gpsimd.reg_load(reg, cnt_s[:1, sc:sc + 1])
                        coff = (int(blk_off[lt, c]) + q) * 8
                        nc.gpsimd.dma_gather(
                            msg[:, moff + q:moff + q + bq, :],
                            tab_ap[c * ch:(c + 1) * ch, :],
                            idx_s[:, coff:coff + bq * 8],
                            bq * 128, reg, feat)
                sel = selp.tile([128, BH_MAX, 128], sel_dtype, tag="sel")
                nc.vector.tensor_tensor(
                    sel[:, :bh, :],
                    iota_t[:].rearrange("p (b m) -> p b m", b=1).to_broadcast((128, bh, 128)),
                    dl_t[:, boff:boff + bh].rearrange("p (b m) -> p b m", m=1).to_broadcast((128, bh, 128)),
                    AOT.is_equal)
                nc.vector.tensor_tensor(
                    sel[:, :bh, :], sel[:, :bh, :],
                    co_t[:, boff:boff + bh].rearrange("p (b m) -> p b m", m=1).to_broadcast((128, bh, 128)),
                    AOT.mult)
                for b in range(bh):
                    nc.tensor.matmul(ps[:], sel[:, b, :], msg[:, b, :],
                                     start=(done == 0), stop=(done == bt_total - 1))
                    done += 1
            # bias + layernorm (+ gelu)
            xb = ln.tile([128, feat], F32, tag="xb")
            r1 = ln.tile([128, 1], F32, tag="r1")
            nc.vector.scalar_tensor_tensor(xb[:], ps[:], 0.0, bias_t[:, 0, :],
                                           AOT.add, AOT.add, accum_out=r1[:])
            sq = ln.tile([128, feat], F32, tag="sq")
            r2 = ln.tile([128, 1], F32, tag="r2")
            nc.scalar.activation(sq[:], xb[:], AFT.Square, accum_out=r2[:])
            mu = ln.tile([128, 1], F32, tag="mu")
            nc.vector.tensor_scalar(mu[:], r1[:], 1.0 / feat, None, AOT.mult)
            musq = ln.tile([128, 1], F32, tag="musq")
            nc.vector.tensor_tensor(musq[:], mu[:], mu[:], AOT.mult)
            var = ln.tile([128, 1], F32, tag="var")
            nc.vector.tensor_scalar(var[:], r2[:], 1.0 / feat, musq[:],
                                    AOT.mult, AOT.subtract)
            st = ln.tile([128, 1], F32, tag="st")
            nc.scalar.activation(st[:], var[:], AFT.Sqrt, bias=eps_t[:])
            rstd = ln.tile([128, 1], F32, tag="rstd")
            nc.vector.reciprocal(rstd[:], st[:])
            xn = ln.tile([128, feat], F32, tag="xn")
            nc.vector.tensor_scalar(xn[:], xb[:], mu[:], rstd[:],
                                    AOT.subtract, AOT.mult)
            y = ln.tile([128, feat], F32, tag="y")
            nc.vector.tensor_tensor(y[:], xn[:], bias_t[:, 1, :], AOT.mult)
            nc.vector.tensor_tensor(y[:], y[:], bias_t[:, 2, :], AOT.add)
            if gelu:
                h = ln.tile([128, feat], F32, tag="h")
                nc.scalar.activation(h[:], y[:], AFT.Gelu)
                out_cb(lt, h)
            else:
                out_cb(lt, y)

    # ---- L1 aggregation; fused stage C (H2 = h1 @ W2) per tile ----
    def l1_out(lt, h):
        h1T = work.tile([128, n_h_ch, 128], F32, tag="h1T")
        for c in range(n_h_ch):
            pst = ps128.tile([128, 128], F32, tag="psT")
            nc.tensor.transpose(pst[:], h[:, c * 128:(c + 1) * 128], ident[:])
            nc.vector.tensor_copy(h1T[:, c, :], pst[:])
        ps2 = ps128.tile([128, HID], F32, tag="psC")
        for c in range(n_h_ch):
            nc.tensor.matmul(ps2[:], h1T[:, c, :], w2s[:, c, :],
                             start=(c == 0), stop=(c == n_h_ch - 1))
        h2 = work.tile([128, HID], F32, tag="h2")
        nc.vector.tensor_copy(h2[:], ps2[:])
        nc.sync.dma_start(ag2_in[lt * 128:(lt + 1) * 128, :], h2[:])

    agg_layer(ag1_out[:], HID2, sel1_dt, co_l1, dl_l1, iota_l1, bias1, True, l1_out)

    nc.gpsimd.collective_compute(
        "AllGather", AOT.bypass,
        replica_groups=[list(range(geom["n_cores"]))],
        ins=[ag2_in.opt()], outs=[ag2_out.opt()])

    # ---- L2 aggregation -> final output ----
    def l2_out(lt, y):
        o = work.tile([128, HID], F32, tag="o")
        nc.vector.tensor_copy(o[:], y[:])
        nc.sync.dma_start(io["out"][lt * 128:(lt + 1) * 128, :], o[:])

    agg_layer(ag2_out[:], HID, F32, co32, dl32, iota32, bias2, False, l2_out)
    ctx.close()


# ============================ top-level kernel ============================

def declare_io(nc, geom, tab1_dt=F32, sel1_dt=F32):
    shard = geom["shard"]
    in_dim = geom["in_dim"]
    NB = geom["NB"]
    io = {
        "xt": nc.dram_tensor("xt", [in_dim, shard], F32, kind="ExternalInput").ap(),
        "w1": nc.dram_tensor("w1", [in_dim, HID2], F32, kind="ExternalInput").ap(),
        "w2": nc.dram_tensor("w2", [HID2, HID], F32, kind="ExternalInput").ap(),
        "bias1": nc.dram_tensor("bias1", [128, 3, HID2], F32, kind="ExternalInput").ap(),
        "bias2": nc.dram_tensor("bias2", [128, 3, HID], F32, kind="ExternalInput").ap(),
        "iota32": nc.dram_tensor("iota32", [128, 128], F32, kind="ExternalInput").ap(),
        "ident": nc.dram_tensor("ident", [128, 128], F32, kind="ExternalInput").ap(),
        "idx": nc.dram_tensor("idx", [128, NB * 8], dt.int16, kind="ExternalInput").ap(),
        "co32": nc.dram_tensor("co32", [128, NB], F32, kind="ExternalInput").ap(),
        "dl32": nc.dram_tensor("dl32", [128, NB], F32, kind="ExternalInput").ap(),
        "cnt": nc.dram_tensor("cnt", [1, geom["n_subcalls"]], dt.int32,
                              kind="ExternalInput").ap(),
        "out": nc.dram_tensor("out", [shard, HID], F32, kind="ExternalOutput").ap(),
    }
    if sel1_dt != F32:
        io["iota_b"] = nc.dram_tensor("iota_b", [128, 128], sel1_dt, kind="ExternalInput").ap()
        io["co_b"] = nc.dram_tensor("co_b", [128, NB], sel1_dt, kind="ExternalInput").ap()
        io["dl_b"] = nc.dram_tensor("dl_b", [128, NB], sel1_dt, kind="ExternalInput").ap()
    return io


def make_host_inputs(geom, per_core, W1, b1, g1, be1, W2, b2, g2, be2, sel1_dt=F32):
    iota_np = np.tile(np.arange(128, dtype=np.float32)[None, :], (128, 1))
    ident_np = np.eye(128, dtype=np.float32)
    bias1_np = np.broadcast_to(
        np.stack([np.asarray(b1, np.float32), np.asarray(g1, np.float32),
                  np.asarray(be1, np.float32)])[None], (128, 3, len(b1))).copy()
    bias2_np = np.broadcast_to(
        np.stack([np.asarray(b2, np.float32), np.asarray(g2, np.float32),
                  np.asarray(be2, np.float32)])[None], (128, 3, len(b2))).copy()
    in_maps = []
    for pc in per_core:
        m = {
            "xt": pc["xt"],
            "w1": np.asarray(W1, np.float32),
            "w2": np.asarray(W2, np.float32),
            "bias1": bias1_np,
            "bias2": bias2_np,
            "iota32": iota_np,
            "ident": ident_np,
            "idx": pc["idx"],
            "co32": pc["co"],
            "dl32": pc["dl"],
            "cnt": pc["cnt"],
        }
        if sel1_dt != F32:
            np_b = dt.np(sel1_dt)
            m["iota_b"] = iota_np.astype(np_b)
            m["co_b"] = pc["co"].astype(np_b)
            m["dl_b"] = pc["dl"].astype(np_b)
        in_maps.append(m)
    return in_maps


def build_nc(geom, tab1_dt=F32, sel1_dt=F32):
    nc = bacc.Bacc("TRN2", debug=False, num_devices=geom["n_cores"])
    io = declare_io(nc, geom, tab1_dt, sel1_dt)
    with tile.TileContext(nc) as tc:
        build_program(tc, io, geom, tab1_dt=tab1_dt, sel1_dt=sel1_dt)
    nc.compile()
    return nc


def kernel(x, edge_index, W1, b1, g1, be1, W2, b2, g2, be2,
           tab1_dt=F32, sel1_dt=F32, trace=False, _return_raw=False):
    x = np.asarray(x, np.float32)
    geom, per_core = preprocess(x, edge_index, N_CORES, TPC)
    nc = build_nc(geom, tab1_dt=tab1_dt, sel1_dt=sel1_dt)
    in_maps = make_host_inputs(geom, per_core, W1, b1, g1, be1, W2, b2, g2, be2,
                               sel1_dt=sel1_dt)
    res = run_bass_kernel_spmd(nc, in_maps, core_ids=list(range(N_CORES)),
                               trace=trace)
    out = np.empty((x.shape[0], HID), np.float32)
    for k, pc in enumerate(per_core):
        ok = np.asarray(res.results[k]["out"])
        out[pc["nodes"]] = ok[pc["pos"]]
    if _return_raw:
        return out, res
    return out



# revision 6
# speedup vs baseline: 1.1919x; 1.1919x over previous
"""Trainium2 Bass kernel for a 2-layer GCN encoder (GCNConv -> LN -> GELU -> GCNConv -> LN).

Strategy (8 NeuronCores, SPMD), v2:
  - Nodes assigned to 784 global tiles of 128 (degree-balanced); core k owns
    tiles t with t%8==k (98 tiles = 12544 dst rows per core).
  - Layer 1: every core computes the FULL transformed table H1*dinv locally
    (X@W1 on all 784 tiles; no collective). Per-core table row order is a
    per-core permutation with the core's own tiles LAST so "my rows" sit at a
    core-independent offset.
  - Aggregation: normalization folded into the table (rows pre-scaled by
    dinv[src]) and the output (post-scaled by dinv[dst]); selector matrices
    are pure one-hot (single is_equal). Self-loops handled by adding the
    node's own table row (sequential read), not as gather edges.
  - Gathers: bf16 rows via dma_gather in fixed 2048-idx calls (16 blocks),
    fully padded (pad idx=0, pad slot=200 -> zero selector column), so no
    per-call count registers and no NaN-guard memsets.
  - Layer 2: transform locally (h1g @ W2, scaled by dinv), AllGather the
    bf16 table (core-major rows), aggregate the same way.
"""

from contextlib import ExitStack

import numpy as np

import concourse.bass as bass
import concourse.bacc as bacc
import concourse.mybir as mybir
import concourse.tile as tile
from concourse.bass_utils import run_bass_kernel_spmd

dt = mybir.dt
F32 = dt.float32
BF16 = dt.bfloat16

# -------- problem geometry (hardcoded for the graded problem) --------
N_FULL = 100000
IN_DIM = 256
HID2 = 256
HID = 128
N_CORES = 8
TILE = 128
NT = 784           # global tiles
TPC = 98           # tiles per core
SHARD = TPC * TILE # 12544
PADN = NT * TILE   # 100352
NCHUNK = 4
CH = PADN // NCHUNK  # 25088 (int16-safe)
CALLB = 8          # blocks per gather call (1024 idxs; 64-desc/engine packet limit)
NI = CALLB * TILE  # 2048
MYBASE = (NT - TPC) * TILE  # 87808: per-core table rows of own tiles
PADSLOT = 200.0


# ============================ bass program builder ============================

def build_program(tc, io, geom):
    nc = tc.nc
    AOT = mybir.AluOpType
    AFT = mybir.ActivationFunctionType
    eps = 1e-5
    n_in_ch = IN_DIM // 128
    n_h_ch = HID2 // 128

    ctx = ExitStack()
    consts = ctx.enter_context(tc.tile_pool(name="consts", bufs=1))
    work = ctx.enter_context(tc.tile_pool(name="work", bufs=2))
    ln = ctx.enter_context(tc.tile_pool(name="ln", bufs=3))
    msgp = ctx.enter_context(tc.tile_pool(name="msgp", bufs=8))
    idxp = ctx.enter_context(tc.tile_pool(name="idxp", bufs=4))
    selp = ctx.enter_context(tc.tile_pool(name="selp", bufs=3))
    trowp = ctx.enter_context(tc.tile_pool(name="trowp", bufs=2))
    ps256 = ctx.enter_context(tc.tile_pool(name="ps256", bufs=2, space="PSUM"))
    ps128 = ctx.enter_context(tc.tile_pool(name="ps128", bufs=2, space="PSUM"))
    dram = ctx.enter_context(tc.tile_pool(name="dram", bufs=1, space="DRAM"))

    # ---- constants ----
    w1s = consts.tile([128, n_in_ch, HID2], BF16)
    nc.sync.dma_start(w1s[:], io["w1"].rearrange("(c p) n -> p c n", p=128))
    w2s = consts.tile([128, n_h_ch, HID], BF16)
    nc.sync.dma_start(w2s[:], io["w2"].rearrange("(c p) n -> p c n", p=128))
    bias1 = consts.tile([128, 3, HID2], F32)
    nc.sync.dma_start(bias1[:], io["bias1"])
    bias2 = consts.tile([128, 3, HID], F32)
    nc.sync.dma_start(bias2[:], io["bias2"])
    ident = consts.tile([128, 128], F32)
    nc.sync.dma_start(ident[:], io["ident"])
    iota_b = consts.tile([128, 128], BF16)
    nc.sync.dma_start(iota_b[:], io["iota_b"])
    dinv_t = consts.tile([128, NT], F32)
    nc.sync.dma_start(dinv_t[:], io["dinv_t"])
    dl1 = consts.tile([128, geom["NB1"]], BF16)
    nc.sync.dma_start(dl1[:], io["dl1"])
    dl2 = consts.tile([128, geom["NB2"]], BF16)
    nc.sync.dma_start(dl2[:], io["dl2"])
    eps_t = consts.tile([128, 1], F32)
    nc.vector.memset(eps_t[:], eps)
    c2048 = consts.tile([1, 1], dt.int32)
    nc.sync.dma_start(c2048[:], io["c2048"])
    r2048 = nc.alloc_register(mybir.EngineType.Pool, "gNI")
    nc.gpsimd.reg_load(r2048, c2048[:1, :1])

    # ---- DRAM buffers ----
    tab1 = dram.tile([PADN, HID2], BF16)
    ag2_in = dram.tile([SHARD, HID], BF16)
    ag2_out = dram.tile([PADN, HID], BF16, addr_space="Shared")

    # ---- stage A: full local table1 = dinv * (X @ W1), bf16 ----
    for t in range(NT):
        xt_t = work.tile([128, n_in_ch, 128], BF16, tag="xt")
        nc.sync.dma_start(
            xt_t[:],
            io["xt"][:, t * 128:(t + 1) * 128].rearrange("(c p) n -> p c n", p=128))
        ps = ps256.tile([128, HID2], F32, tag="psA")
        for cc in range(n_in_ch):
            nc.tensor.matmul(ps[:], xt_t[:, cc, :], w1s[:, cc, :],
                             start=(cc == 0), stop=(cc == n_in_ch - 1))
        h1t = work.tile([128, HID2], BF16, tag="h1t")
        nc.scalar.activation(h1t[:], ps[:], AFT.Copy, scale=dinv_t[:, t:t + 1])
        nc.sync.dma_start(tab1[t * 128:(t + 1) * 128, :], h1t[:])

    # ---- generic aggregation layer ----
    def agg_layer(tab_ap, feat, B, S, CB, NC, dl_t, io_idx, bias_t, gelu, trow_src, out_cb):
        # emit all gather calls (Tile pipelines via pool WAR deps)
        msg_tiles = {}
        maxw = int(max(NC))
        for w in range(maxw):
            for cc in range(NCHUNK):
                if w >= NC[cc]:
                    continue
                it = idxp.tile([128, NI // 16], dt.int16, tag="idx")
                col0 = int(CB[cc] + w * CALLB) * 8
                nc.sync.dma_start(it[:], io_idx[:, col0:col0 + NI // 16])
                msg = msgp.tile([128, CALLB, feat], BF16, tag="msg")
                nc.gpsimd.dma_gather(
                    msg[:], tab_ap[cc * CH:(cc + 1) * CH, :], it[:],
                    NI, r2048, feat)
                msg_tiles[(cc, w)] = msg

        for lt in range(TPC):
            bt = int(B[lt].sum())
            assert bt > 0
            ps = ps256.tile([128, feat], F32, tag="psAgg")
            done = 0
            for cc in range(NCHUNK):
                bc = int(B[lt, cc])
                if bc == 0:
                    continue
                sel = selp.tile([128, bc, 128], BF16, tag="sel")
                g0 = int(CB[cc] + S[lt, cc])
                nc.vector.tensor_tensor(
                    sel[:],
                    iota_b[:].rearrange("p (b m) -> p b m", b=1).to_broadcast((128, bc, 128)),
                    dl_t[:, g0:g0 + bc].rearrange("p (b m) -> p b m", m=1).to_broadcast((128, bc, 128)),
                    AOT.is_equal)
                for bi in range(bc):
                    w, j = divmod(g0 + bi, CALLB)
                    w -= int(CB[cc]) // CALLB
                    msg = msg_tiles[(cc, w)]
                    nc.tensor.matmul(ps[:], sel[:, bi, :], msg[:, j, :],
                                     start=(done == 0), stop=(done == bt - 1))
                    done += 1
            # + own row (self loop), scale by dinv[dst], +bias, LN (+gelu)
            trow = trowp.tile([128, feat], BF16, tag="trow")
            nc.sync.dma_start(trow[:], trow_src[lt * 128:(lt + 1) * 128, :])
            t_mine = (NT - TPC) + lt  # position of my lt-th tile in per-core order
            xbA = ln.tile([128, feat], F32, tag="xbA")
            nc.vector.tensor_tensor(xbA[:], ps[:], trow[:], AOT.add)
            xb = ln.tile([128, feat], F32, tag="xb")
            r1 = ln.tile([128, 1], F32, tag="r1")
            nc.vector.scalar_tensor_tensor(xb[:], xbA[:], dinv_t[:, t_mine:t_mine + 1],
                                           bias_t[:, 0, :], AOT.mult, AOT.add,
                                           accum_out=r1[:])
            sq = ln.tile([128, feat], F32, tag="sq")
            r2 = ln.tile([128, 1], F32, tag="r2")
            nc.scalar.activation(sq[:], xb[:], AFT.Square, accum_out=r2[:])
            mu = ln.tile([128, 1], F32, tag="mu")
            nc.vector.tensor_scalar(mu[:], r1[:], 1.0 / feat, None, AOT.mult)
            musq = ln.tile([128, 1], F32, tag="musq")
            nc.vector.tensor_tensor(musq[:], mu[:], mu[:], AOT.mult)
            var = ln.tile([128, 1], F32, tag="var")
            nc.vector.tensor_scalar(var[:], r2[:], 1.0 / feat, musq[:],
                                    AOT.mult, AOT.subtract)
            st = ln.tile([128, 1], F32, tag="st")
            nc.scalar.activation(st[:], var[:], AFT.Sqrt, bias=eps_t[:])
            rstd = ln.tile([128, 1], F32, tag="rstd")
            nc.vector.reciprocal(rstd[:], st[:])
            xn = ln.tile([128, feat], F32, tag="xn")
            nc.vector.tensor_scalar(xn[:], xb[:], mu[:], rstd[:],
                                    AOT.subtract, AOT.mult)
            y = ln.tile([128, feat], F32, tag="y")
            nc.vector.tensor_tensor(y[:], xn[:], bias_t[:, 1, :], AOT.mult)
            nc.vector.tensor_tensor(y[:], y[:], bias_t[:, 2, :], AOT.add)
            if gelu:
                h = ln.tile([128, feat], F32, tag="h")
                nc.scalar.activation(h[:], y[:], AFT.Gelu)
                out_cb(lt, h)
            else:
                out_cb(lt, y)

    # ---- L1 -> transform to table2 rows (dinv * h1g @ W2) ----
    def l1_out(lt, h):
        t_mine = (NT - TPC) + lt
        h1T = work.tile([128, n_h_ch, 128], BF16, tag="h1T")
        for cc in range(n_h_ch):
            pst = ps128.tile([128, 128], F32, tag="psT")
            nc.tensor.transpose(pst[:], h[:, cc * 128:(cc + 1) * 128], ident[:])
            nc.vector.tensor_copy(h1T[:, cc, :], pst[:])
        ps2 = ps128.tile([128, HID], F32, tag="psC")
        for cc in range(n_h_ch):
            nc.tensor.matmul(ps2[:], h1T[:, cc, :], w2s[:, cc, :],
                             start=(cc == 0), stop=(cc == n_h_ch - 1))
        h2 = work.tile([128, HID], BF16, tag="h2")
        nc.scalar.activation(h2[:], ps2[:], AFT.Copy, scale=dinv_t[:, t_mine:t_mine + 1])
        nc.sync.dma_start(ag2_in[lt * 128:(lt + 1) * 128, :], h2[:])

    agg_layer(tab1[:], HID2, geom["B1"], geom["S1"], geom["CB1"], geom["NC1"],
              dl1, io["idx1"], bias1, True, tab1[MYBASE:, :], l1_out)

    nc.gpsimd.collective_compute(
        "AllGather", AOT.bypass,
        replica_groups=[list(range(N_CORES))],
        ins=[ag2_in.opt()], outs=[ag2_out.opt()])

    # ---- L2 aggregation -> final output ----
    def l2_out(lt, y):
        o = work.tile([128, HID], F32, tag="o")
        nc.vector.tensor_copy(o[:], y[:])
        nc.sync.dma_start(io["out"][lt * 128:(lt + 1) * 128, :], o[:])

    agg_layer(ag2_out[:], HID, geom["B2"], geom["S2"], geom["CB2"], geom["NC2"],
              dl2, io["idx2"], bias2, False, ag2_in[:], l2_out)
    ctx.close()


# ============================ top-level kernel ============================

def declare_io(nc, geom):
    io = {
        "xt": nc.dram_tensor("xt", [IN_DIM, PADN], BF16, kind="ExternalInput").ap(),
        "w1": nc.dram_tensor("w1", [IN_DIM, HID2], BF16, kind="ExternalInput").ap(),
        "w2": nc.dram_tensor("w2", [HID2, HID], BF16, kind="ExternalInput").ap(),
        "bias1": nc.dram_tensor("bias1", [128, 3, HID2], F32, kind="ExternalInput").ap(),
        "bias2": nc.dram_tensor("bias2", [128, 3, HID], F32, kind="ExternalInput").ap(),
        "iota_b": nc.dram_tensor("iota_b", [128, 128], BF16, kind="ExternalInput").ap(),
        "ident": nc.dram_tensor("ident", [128, 128], F32, kind="ExternalInput").ap(),
        "dinv_t": nc.dram_tensor("dinv_t", [128, NT], F32, kind="ExternalInput").ap(),
        "idx1": nc.dram_tensor("idx1", [128, geom["NB1"] * 8], dt.int16,
                               kind="ExternalInput").ap(),
        "dl1": nc.dram_tensor("dl1", [128, geom["NB1"]], BF16, kind="ExternalInput").ap(),
        "idx2": nc.dram_tensor("idx2", [128, geom["NB2"] * 8], dt.int16,
                               kind="ExternalInput").ap(),
        "dl2": nc.dram_tensor("dl2", [128, geom["NB2"]], BF16, kind="ExternalInput").ap(),
        "c2048": nc.dram_tensor("c2048", [1, 1], dt.int32, kind="ExternalInput").ap(),
        "out": nc.dram_tensor("out", [SHARD, HID], F32, kind="ExternalOutput").ap(),
    }
    return io


def kernel(x, edge_index, W1, b1, g1, be1, W2, b2, g2, be2,
           trace=False, _return_raw=False):
    bf = dt.np(BF16)
    x = np.asarray(x, np.float32)
    src = np.asarray(edge_index[0], np.int64)
    dst = np.asarray(edge_index[1], np.int64)
    N = x.shape[0]

    deg = (np.bincount(dst, minlength=N) + 1).astype(np.float32)
    dinv = (1.0 / np.sqrt(deg)).astype(np.float32)

    order = np.argsort(-deg, kind="stable")
    node_tile = np.empty(N, np.int32)
    node_slot = np.empty(N, np.int32)
    ar = np.arange(N, dtype=np.int64)
    node_tile[order] = (ar % NT).astype(np.int32)
    node_slot[order] = (ar // NT).astype(np.int32)
    core_of = node_tile % N_CORES
    lt_of = node_tile // N_CORES

    dinv_st = np.ones((TILE, NT), np.float32)
    dinv_st[node_slot, node_tile] = dinv
    row2 = core_of.astype(np.int64) * SHARD + lt_of.astype(np.int64) * TILE + node_slot

    # --- per-core packing ---
    cores = []
    cnts1, cnts2 = [], []
    for k in range(N_CORES):
        others = np.setdiff1d(np.arange(NT, dtype=np.int64),
                              np.arange(k, NT, N_CORES, dtype=np.int64),
                              assume_unique=True)
        mine = np.arange(k, NT, N_CORES, dtype=np.int64)
        tord = np.concatenate([others, mine])
        tpos = np.empty(NT, np.int64)
        tpos[tord] = np.arange(NT, dtype=np.int64)
        row1 = tpos[node_tile] * TILE + node_slot

        m = core_of[dst] == k
        elt = lt_of[dst[m]].astype(np.int64)
        eslot = node_slot[dst[m]].astype(np.float32)
        esrc = src[m]

        def sort_pack(srcrow):
            c = srcrow // CH
            i16 = (srcrow - c * CH).astype(np.int16)
            key = elt * NCHUNK + c
            o = np.argsort(key, kind="stable")
            cnts = np.bincount(key, minlength=TPC * NCHUNK).reshape(TPC, NCHUNK)
            return i16[o], eslot[o], cnts

        i16a, sla, ca = sort_pack(row1[esrc])
        i16b, slb, cb = sort_pack(row2[esrc])
        cnts1.append(ca)
        cnts2.append(cb)

        xs = np.zeros((PADN, IN_DIM), np.float32)
        xs[row1] = x
        cores.append(dict(
            xt=np.ascontiguousarray(xs.T).astype(bf),
            dinv_t=np.ascontiguousarray(dinv_st[:, tord]),
            e1=(i16a, sla, ca), e2=(i16b, slb, cb),
            nodes=np.nonzero(core_of == k)[0]))

    B1, S1, CB1, NC1, NB1 = finalize_geometry(cnts1)
    B2, S2, CB2, NC2, NB2 = finalize_geometry(cnts2)
    geom = dict(B1=B1, S1=S1, CB1=CB1, NC1=NC1, NB1=NB1,
                B2=B2, S2=S2, CB2=CB2, NC2=NC2, NB2=NB2)

    iota_np = np.tile(np.arange(128, dtype=np.float32)[None, :], (128, 1)).astype(bf)
    ident_np = np.eye(128, dtype=np.float32)
    bias1_np = np.broadcast_to(
        np.stack([np.asarray(b1, np.float32), np.asarray(g1, np.float32),
                  np.asarray(be1, np.float32)])[None], (128, 3, HID2)).copy()
    bias2_np = np.broadcast_to(
        np.stack([np.asarray(b2, np.float32), np.asarray(g2, np.float32),
                  np.asarray(be2, np.float32)])[None], (128, 3, HID)).copy()

    in_maps = []
    for k in range(N_CORES):
        pc = cores[k]
        idx1, dl1 = build_core_arrays(pc["e1"], B1, S1, CB1, NB1)
        idx2, dl2 = build_core_arrays(pc["e2"], B2, S2, CB2, NB2)
        in_maps.append({
            "xt": pc["xt"], "w1": np.asarray(W1, np.float32).astype(bf),
            "w2": np.asarray(W2, np.float32).astype(bf),
            "bias1": bias1_np, "bias2": bias2_np,
            "iota_b": iota_np, "ident": ident_np,
            "dinv_t": pc["dinv_t"],
            "idx1": idx1, "dl1": dl1, "idx2": idx2, "dl2": dl2,
            "c2048": np.array([[NI]], np.int32),
        })

    nc = bacc.Bacc("TRN2", debug=False, num_devices=N_CORES)
    io = declare_io(nc, geom)
    with tile.TileContext(nc) as tc:
        build_program(tc, io, geom)
    nc.compile()

    res = run_bass_kernel_spmd(nc, in_maps, core_ids=list(range(N_CORES)),
                               trace=trace)
    out = np.empty((N, HID), np.float32)
    for k in range(N_CORES):
        pc = cores[k]
        ok = np.asarray(res.results[k]["out"])
        pos = lt_of[pc["nodes"]] * TILE + node_slot[pc["nodes"]]
        out[pc["nodes"]] = ok[pos]
    if _return_raw:
        return out, res
    return out


def build_core_arrays(epack, B, S, CB, NB):
    bf = dt.np(BF16)
    i16, slot, cnts = epack
    idx_a = np.zeros((16, NB * 8), np.int16)
    dl_a = np.full((TILE, NB), PADSLOT, np.float32)
    starts = np.zeros(TPC * NCHUNK + 1, np.int64)
    np.cumsum(cnts.reshape(-1), out=starts[1:])
    for lt in range(TPC):
        for cc in range(NCHUNK):
            m = int(cnts[lt, cc])
            if m == 0:
                continue
            s0 = int(starts[lt * NCHUNK + cc])
            p = (int(CB[cc] + S[lt, cc])) * TILE + np.arange(m)
            idx_a[p % 16, p // 16] = i16[s0:s0 + m]
            dl_a[p % TILE, p // TILE] = slot[s0:s0 + m]
    return np.tile(idx_a, (8, 1)), dl_a.astype(bf)


def finalize_geometry(cnts_list):
    allc = np.stack(cnts_list)  # [8, TPC, NCHUNK]
    B = (-(-allc.max(axis=0) // TILE)).astype(np.int64)
    S = np.zeros((TPC, NCHUNK), np.int64)
    CB = np.zeros(NCHUNK + 1, np.int64)
    NC = np.zeros(NCHUNK, np.int64)
    for cc in range(NCHUNK):
        S[:, cc] = np.cumsum(B[:, cc]) - B[:, cc]
        nb = int(B[:, cc].sum())
        NC[cc] = -(-nb // CALLB)
        CB[cc + 1] = CB[cc] + NC[cc] * CALLB
    return B, S, CB, NC, int(CB[NCHUNK])


# revision 8
# speedup vs baseline: 1.2294x; 1.0315x over previous
"""Trainium2 Bass kernel for a 2-layer GCN encoder (GCNConv -> LN -> GELU -> GCNConv -> LN).

Strategy (8 NeuronCores, SPMD), v2:
  - Nodes assigned to 784 global tiles of 128 (degree-balanced); core k owns
    tiles t with t%8==k (98 tiles = 12544 dst rows per core).
  - Layer 1: every core computes the FULL transformed table H1*dinv locally
    (X@W1 on all 784 tiles; no collective). Per-core table row order is a
    per-core permutation with the core's own tiles LAST so "my rows" sit at a
    core-independent offset.
  - Aggregation: normalization folded into the table (rows pre-scaled by
    dinv[src]) and the output (post-scaled by dinv[dst]); selector matrices
    are pure one-hot (single is_equal). Self-loops handled by adding the
    node's own table row (sequential read), not as gather edges.
  - Gathers: bf16 rows via dma_gather in fixed 2048-idx calls (16 blocks),
    fully padded (pad idx=0, pad slot=200 -> zero selector column), so no
    per-call count registers and no NaN-guard memsets.
  - Layer 2: transform locally (h1g @ W2, scaled by dinv), AllGather the
    bf16 table (core-major rows), aggregate the same way.
"""

from contextlib import ExitStack

import numpy as np

import concourse.bass as bass
import concourse.bacc as bacc
import concourse.mybir as mybir
import concourse.tile as tile
from concourse.bass_utils import run_bass_kernel_spmd

dt = mybir.dt
F32 = dt.float32
BF16 = dt.bfloat16

# -------- problem geometry (hardcoded for the graded problem) --------
N_FULL = 100000
IN_DIM = 256
HID2 = 256
HID = 128
N_CORES = 8
TILE = 128
NT = 784           # global tiles
TPC = 98           # tiles per core
SHARD = TPC * TILE # 12544
PADN = NT * TILE   # 100352
NCHUNK = 4
CH = PADN // NCHUNK  # 25088 (int16-safe)
CALLB = 16         # blocks per gather call (2048 idxs; multi-packet)
NI = CALLB * TILE  # 2048
MYBASE = (NT - TPC) * TILE  # 87808: per-core table rows of own tiles
PADSLOT = 200.0


# ============================ bass program builder ============================

def build_program(tc, io, geom):
    nc = tc.nc
    AOT = mybir.AluOpType
    AFT = mybir.ActivationFunctionType
    eps = 1e-5
    n_in_ch = IN_DIM // 128
    n_h_ch = HID2 // 128

    ctx = ExitStack()
    consts = ctx.enter_context(tc.tile_pool(name="consts", bufs=1))
    work = ctx.enter_context(tc.tile_pool(name="work", bufs=2))
    ln = ctx.enter_context(tc.tile_pool(name="ln", bufs=3))
    msgp = ctx.enter_context(tc.tile_pool(name="msgp", bufs=8))
    idxp = ctx.enter_context(tc.tile_pool(name="idxp", bufs=4))
    selp = ctx.enter_context(tc.tile_pool(name="selp", bufs=3))
    trowp = ctx.enter_context(tc.tile_pool(name="trowp", bufs=2))
    ps256 = ctx.enter_context(tc.tile_pool(name="ps256", bufs=2, space="PSUM"))
    ps128 = ctx.enter_context(tc.tile_pool(name="ps128", bufs=2, space="PSUM"))
    dram = ctx.enter_context(tc.tile_pool(name="dram", bufs=1, space="DRAM"))

    # ---- constants ----
    w1s = consts.tile([128, n_in_ch, HID2], BF16)
    nc.sync.dma_start(w1s[:], io["w1"].rearrange("(c p) n -> p c n", p=128))
    w2s = consts.tile([128, n_h_ch, HID], BF16)
    nc.sync.dma_start(w2s[:], io["w2"].rearrange("(c p) n -> p c n", p=128))
    bias1 = consts.tile([128, 3, HID2], F32)
    nc.sync.dma_start(bias1[:], io["bias1"])
    bias2 = consts.tile([128, 3, HID], F32)
    nc.sync.dma_start(bias2[:], io["bias2"])
    ident = consts.tile([128, 128], F32)
    nc.sync.dma_start(ident[:], io["ident"])
    iota_b = consts.tile([128, 128], BF16)
    nc.sync.dma_start(iota_b[:], io["iota_b"])
    dinv_t = consts.tile([128, NT], F32)
    nc.sync.dma_start(dinv_t[:], io["dinv_t"])
    dl1 = consts.tile([128, geom["NB1"]], BF16)
    nc.sync.dma_start(dl1[:], io["dl1"])
    dl2 = consts.tile([128, geom["NB2"]], BF16)
    nc.sync.dma_start(dl2[:], io["dl2"])
    eps_t = consts.tile([128, 1], F32)
    nc.vector.memset(eps_t[:], eps)
    c2048 = consts.tile([1, 1], dt.int32)
    nc.sync.dma_start(c2048[:], io["c2048"])
    r2048 = nc.alloc_register(mybir.EngineType.Pool, "gNI")
    nc.gpsimd.reg_load(r2048, c2048[:1, :1])

    # ---- DRAM buffers ----
    tab1c = [dram.tile([CH, HID2], BF16, name=f"tab1c{i}") for i in range(NCHUNK)]
    ag2_in = dram.tile([SHARD, HID], BF16)
    ag2_out = dram.tile([PADN, HID], BF16, addr_space="Shared")

    # ---- stage A: full local table1 = dinv * (X @ W1), bf16 ----
    for t in range(NT):
        xt_t = work.tile([128, n_in_ch, 128], BF16, tag="xt")
        nc.sync.dma_start(
            xt_t[:],
            io["xt"][:, t * 128:(t + 1) * 128].rearrange("(c p) n -> p c n", p=128))
        ps = ps256.tile([128, HID2], F32, tag="psA")
        for cc in range(n_in_ch):
            nc.tensor.matmul(ps[:], xt_t[:, cc, :], w1s[:, cc, :],
                             start=(cc == 0), stop=(cc == n_in_ch - 1))
        h1t = work.tile([128, HID2], BF16, tag="h1t")
        nc.scalar.activation(h1t[:], ps[:], AFT.Copy, scale=dinv_t[:, t:t + 1])
        tc_, tr = divmod(t * 128, CH)
        nc.sync.dma_start(tab1c[tc_][tr:tr + 128, :], h1t[:])

    # ---- generic aggregation layer ----
    def agg_layer(tab_list, feat, B, S, CB, NC, dl_t, io_idx, bias_t, gelu, trow_src, out_cb):
        # emit all gather calls (Tile pipelines via pool WAR deps)
        msg_tiles = {}
        maxw = int(max(NC))
        for w in range(maxw):
            for cc in range(NCHUNK):
                if w >= NC[cc]:
                    continue
                it = idxp.tile([128, NI // 16], dt.int16, tag="idx")
                col0 = int(CB[cc] + w * CALLB) * 8
                nc.sync.dma_start(it[:], io_idx[:, col0:col0 + NI // 16])
                msg = msgp.tile([128, CALLB, feat], BF16, tag="msg")
                nc.gpsimd.dma_gather(
                    msg[:], tab_list[cc][:], it[:],
                    NI, r2048, feat, single_packet=False)
                msg_tiles[(cc, w)] = msg

        for lt in range(TPC):
            bt = int(B[lt].sum())
            assert bt > 0
            ps = ps256.tile([128, feat], F32, tag="psAgg")
            done = 0
            for cc in range(NCHUNK):
                bc = int(B[lt, cc])
                if bc == 0:
                    continue
                sel = selp.tile([128, bc, 128], BF16, tag="sel")
                g0 = int(CB[cc] + S[lt, cc])
                nc.vector.tensor_tensor(
                    sel[:],
                    iota_b[:].rearrange("p (b m) -> p b m", b=1).to_broadcast((128, bc, 128)),
                    dl_t[:, g0:g0 + bc].rearrange("p (b m) -> p b m", m=1).to_broadcast((128, bc, 128)),
                    AOT.is_equal)
                for bi in range(bc):
                    w, j = divmod(g0 + bi, CALLB)
                    w -= int(CB[cc]) // CALLB
                    msg = msg_tiles[(cc, w)]
                    nc.tensor.matmul(ps[:], sel[:, bi, :], msg[:, j, :],
                                     start=(done == 0), stop=(done == bt - 1))
                    done += 1
            # + own row (self loop), scale by dinv[dst], +bias, LN (+gelu)
            trow = trowp.tile([128, feat], BF16, tag="trow")
            nc.sync.dma_start(trow[:], trow_src[lt * 128:(lt + 1) * 128, :])
            t_mine = (NT - TPC) + lt  # position of my lt-th tile in per-core order
            xbA = ln.tile([128, feat], F32, tag="xbA")
            nc.vector.tensor_tensor(xbA[:], ps[:], trow[:], AOT.add)
            xb = ln.tile([128, feat], F32, tag="xb")
            r1 = ln.tile([128, 1], F32, tag="r1")
            nc.vector.scalar_tensor_tensor(xb[:], xbA[:], dinv_t[:, t_mine:t_mine + 1],
                                           bias_t[:, 0, :], AOT.mult, AOT.add,
                                           accum_out=r1[:])
            sq = ln.tile([128, feat], F32, tag="sq")
            r2 = ln.tile([128, 1], F32, tag="r2")
            nc.scalar.activation(sq[:], xb[:], AFT.Square, accum_out=r2[:])
            mu = ln.tile([128, 1], F32, tag="mu")
            nc.vector.tensor_scalar(mu[:], r1[:], 1.0 / feat, None, AOT.mult)
            musq = ln.tile([128, 1], F32, tag="musq")
            nc.vector.tensor_tensor(musq[:], mu[:], mu[:], AOT.mult)
            var = ln.tile([128, 1], F32, tag="var")
            nc.vector.tensor_scalar(var[:], r2[:], 1.0 / feat, musq[:],
                                    AOT.mult, AOT.subtract)
            st = ln.tile([128, 1], F32, tag="st")
            nc.scalar.activation(st[:], var[:], AFT.Sqrt, bias=eps_t[:])
            rstd = ln.tile([128, 1], F32, tag="rstd")
            nc.vector.reciprocal(rstd[:], st[:])
            xn = ln.tile([128, feat], F32, tag="xn")
            nc.vector.tensor_scalar(xn[:], xb[:], mu[:], rstd[:],
                                    AOT.subtract, AOT.mult)
            y = ln.tile([128, feat], F32, tag="y")
            nc.vector.tensor_tensor(y[:], xn[:], bias_t[:, 1, :], AOT.mult)
            nc.vector.tensor_tensor(y[:], y[:], bias_t[:, 2, :], AOT.add)
            if gelu:
                h = ln.tile([128, feat], F32, tag="h")
                nc.scalar.activation(h[:], y[:], AFT.Gelu)
                out_cb(lt, h)
            else:
                out_cb(lt, y)

    # ---- L1 -> transform to table2 rows (dinv * h1g @ W2) ----
    def l1_out(lt, h):
        t_mine = (NT - TPC) + lt
        h1T = work.tile([128, n_h_ch, 128], BF16, tag="h1T")
        for cc in range(n_h_ch):
            pst = ps128.tile([128, 128], F32, tag="psT")
            nc.tensor.transpose(pst[:], h[:, cc * 128:(cc + 1) * 128], ident[:])
            nc.vector.tensor_copy(h1T[:, cc, :], pst[:])
        ps2 = ps128.tile([128, HID], F32, tag="psC")
        for cc in range(n_h_ch):
            nc.tensor.matmul(ps2[:], h1T[:, cc, :], w2s[:, cc, :],
                             start=(cc == 0), stop=(cc == n_h_ch - 1))
        h2 = work.tile([128, HID], BF16, tag="h2")
        nc.scalar.activation(h2[:], ps2[:], AFT.Copy, scale=dinv_t[:, t_mine:t_mine + 1])
        nc.sync.dma_start(ag2_in[lt * 128:(lt + 1) * 128, :], h2[:])

    agg_layer(tab1c, HID2, geom["B1"], geom["S1"], geom["CB1"], geom["NC1"],
              dl1, io["idx1"], bias1, True, tab1c[3][MYBASE - 3 * CH:, :], l1_out)

    nc.gpsimd.collective_compute(
        "AllGather", AOT.bypass,
        replica_groups=[list(range(N_CORES))],
        ins=[ag2_in.opt()], outs=[ag2_out.opt()])

    # ---- L2 aggregation -> final output ----
    def l2_out(lt, y):
        o = work.tile([128, HID], F32, tag="o")
        nc.vector.tensor_copy(o[:], y[:])
        nc.sync.dma_start(io["out"][lt * 128:(lt + 1) * 128, :], o[:])

    agg_layer([ag2_out[cc * CH:(cc + 1) * CH, :] for cc in range(NCHUNK)],
              HID, geom["B2"], geom["S2"], geom["CB2"], geom["NC2"],
              dl2, io["idx2"], bias2, False, ag2_in[:], l2_out)
    ctx.close()


# ============================ top-level kernel ============================

def declare_io(nc, geom):
    io = {
        "xt": nc.dram_tensor("xt", [IN_DIM, PADN], BF16, kind="ExternalInput").ap(),
        "w1": nc.dram_tensor("w1", [IN_DIM, HID2], BF16, kind="ExternalInput").ap(),
        "w2": nc.dram_tensor("w2", [HID2, HID], BF16, kind="ExternalInput").ap(),
        "bias1": nc.dram_tensor("bias1", [128, 3, HID2], F32, kind="ExternalInput").ap(),
        "bias2": nc.dram_tensor("bias2", [128, 3, HID], F32, kind="ExternalInput").ap(),
        "iota_b": nc.dram_tensor("iota_b", [128, 128], BF16, kind="ExternalInput").ap(),
        "ident": nc.dram_tensor("ident", [128, 128], F32, kind="ExternalInput").ap(),
        "dinv_t": nc.dram_tensor("dinv_t", [128, NT], F32, kind="ExternalInput").ap(),
        "idx1": nc.dram_tensor("idx1", [128, geom["NB1"] * 8], dt.int16,
                               kind="ExternalInput").ap(),
        "dl1": nc.dram_tensor("dl1", [128, geom["NB1"]], BF16, kind="ExternalInput").ap(),
        "idx2": nc.dram_tensor("idx2", [128, geom["NB2"] * 8], dt.int16,
                               kind="ExternalInput").ap(),
        "dl2": nc.dram_tensor("dl2", [128, geom["NB2"]], BF16, kind="ExternalInput").ap(),
        "c2048": nc.dram_tensor("c2048", [1, 1], dt.int32, kind="ExternalInput").ap(),
        "out": nc.dram_tensor("out", [SHARD, HID], F32, kind="ExternalOutput").ap(),
    }
    return io


def kernel(x, edge_index, W1, b1, g1, be1, W2, b2, g2, be2,
           trace=False, _return_raw=False):
    bf = dt.np(BF16)
    x = np.asarray(x, np.float32)
    src = np.asarray(edge_index[0], np.int64)
    dst = np.asarray(edge_index[1], np.int64)
    N = x.shape[0]

    deg = (np.bincount(dst, minlength=N) + 1).astype(np.float32)
    dinv = (1.0 / np.sqrt(deg)).astype(np.float32)

    order = np.argsort(-deg, kind="stable")
    node_tile = np.empty(N, np.int32)
    node_slot = np.empty(N, np.int32)
    ar = np.arange(N, dtype=np.int64)
    node_tile[order] = (ar % NT).astype(np.int32)
    node_slot[order] = (ar // NT).astype(np.int32)
    core_of = node_tile % N_CORES
    lt_of = node_tile // N_CORES

    dinv_st = np.ones((TILE, NT), np.float32)
    dinv_st[node_slot, node_tile] = dinv
    row2 = core_of.astype(np.int64) * SHARD + lt_of.astype(np.int64) * TILE + node_slot

    # --- per-core packing ---
    cores = []
    cnts1, cnts2 = [], []
    for k in range(N_CORES):
        others = np.setdiff1d(np.arange(NT, dtype=np.int64),
                              np.arange(k, NT, N_CORES, dtype=np.int64),
                              assume_unique=True)
        mine = np.arange(k, NT, N_CORES, dtype=np.int64)
        tord = np.concatenate([others, mine])
        tpos = np.empty(NT, np.int64)
        tpos[tord] = np.arange(NT, dtype=np.int64)
        row1 = tpos[node_tile] * TILE + node_slot

        m = core_of[dst] == k
        elt = lt_of[dst[m]].astype(np.int64)
        eslot = node_slot[dst[m]].astype(np.float32)
        esrc = src[m]

        def sort_pack(srcrow):
            c = srcrow // CH
            i16 = (srcrow - c * CH).astype(np.int16)
            key = elt * NCHUNK + c
            o = np.argsort(key, kind="stable")
            cnts = np.bincount(key, minlength=TPC * NCHUNK).reshape(TPC, NCHUNK)
            return i16[o], eslot[o], cnts

        i16a, sla, ca = sort_pack(row1[esrc])
        i16b, slb, cb = sort_pack(row2[esrc])
        cnts1.append(ca)
        cnts2.append(cb)

        xs = np.zeros((PADN, IN_DIM), np.float32)
        xs[row1] = x
        cores.append(dict(
            xt=np.ascontiguousarray(xs.T).astype(bf),
            dinv_t=np.ascontiguousarray(dinv_st[:, tord]),
            e1=(i16a, sla, ca), e2=(i16b, slb, cb),
            nodes=np.nonzero(core_of == k)[0]))

    B1, S1, CB1, NC1, NB1 = finalize_geometry(cnts1)
    B2, S2, CB2, NC2, NB2 = finalize_geometry(cnts2)
    geom = dict(B1=B1, S1=S1, CB1=CB1, NC1=NC1, NB1=NB1,
                B2=B2, S2=S2, CB2=CB2, NC2=NC2, NB2=NB2)

    iota_np = np.tile(np.arange(128, dtype=np.float32)[None, :], (128, 1)).astype(bf)
    ident_np = np.eye(128, dtype=np.float32)
    bias1_np = np.broadcast_to(
        np.stack([np.asarray(b1, np.float32), np.asarray(g1, np.float32),
                  np.asarray(be1, np.float32)])[None], (128, 3, HID2)).copy()
    bias2_np = np.broadcast_to(
        np.stack([np.asarray(b2, np.float32), np.asarray(g2, np.float32),
                  np.asarray(be2, np.float32)])[None], (128, 3, HID)).copy()

    in_maps = []
    for k in range(N_CORES):
        pc = cores[k]
        idx1, dl1 = build_core_arrays(pc["e1"], B1, S1, CB1, NB1)
        idx2, dl2 = build_core_arrays(pc["e2"], B2, S2, CB2, NB2)
        in_maps.append({
            "xt": pc["xt"], "w1": np.asarray(W1, np.float32).astype(bf),
            "w2": np.asarray(W2, np.float32).astype(bf),
            "bias1": bias1_np, "bias2": bias2_np,
            "iota_b": iota_np, "ident": ident_np,
            "dinv_t": pc["dinv_t"],
            "idx1": idx1, "dl1": dl1, "idx2": idx2, "dl2": dl2,
            "c2048": np.array([[NI]], np.int32),
        })

    nc = bacc.Bacc("TRN2", debug=False, num_devices=N_CORES)
    io = declare_io(nc, geom)
    with tile.TileContext(nc) as tc:
        build_program(tc, io, geom)
    nc.compile()

    res = run_bass_kernel_spmd(nc, in_maps, core_ids=list(range(N_CORES)),
                               trace=trace)
    out = np.empty((N, HID), np.float32)
    for k in range(N_CORES):
        pc = cores[k]
        ok = np.asarray(res.results[k]["out"])
        pos = lt_of[pc["nodes"]] * TILE + node_slot[pc["nodes"]]
        out[pc["nodes"]] = ok[pos]
    if _return_raw:
        return out, res
    return out


def build_core_arrays(epack, B, S, CB, NB):
    bf = dt.np(BF16)
    i16, slot, cnts = epack
    idx_a = np.zeros((16, NB * 8), np.int16)
    dl_a = np.full((TILE, NB), PADSLOT, np.float32)
    starts = np.zeros(TPC * NCHUNK + 1, np.int64)
    np.cumsum(cnts.reshape(-1), out=starts[1:])
    for lt in range(TPC):
        for cc in range(NCHUNK):
            m = int(cnts[lt, cc])
            if m == 0:
                continue
            s0 = int(starts[lt * NCHUNK + cc])
            p = (int(CB[cc] + S[lt, cc])) * TILE + np.arange(m)
            idx_a[p % 16, p // 16] = i16[s0:s0 + m]
            dl_a[p % TILE, p // TILE] = slot[s0:s0 + m]
    return np.tile(idx_a, (8, 1)), dl_a.astype(bf)


def finalize_geometry(cnts_list):
    allc = np.stack(cnts_list)  # [8, TPC, NCHUNK]
    B = (-(-allc.max(axis=0) // TILE)).astype(np.int64)
    S = np.zeros((TPC, NCHUNK), np.int64)
    CB = np.zeros(NCHUNK + 1, np.int64)
    NC = np.zeros(NCHUNK, np.int64)
    for cc in range(NCHUNK):
        S[:, cc] = np.cumsum(B[:, cc]) - B[:, cc]
        nb = int(B[:, cc].sum())
        NC[cc] = -(-nb // CALLB)
        CB[cc + 1] = CB[cc] + NC[cc] * CALLB
    return B, S, CB, NC, int(CB[NCHUNK])


# revision 9
# speedup vs baseline: 1.3262x; 1.0787x over previous
"""Trainium2 Bass kernel for a 2-layer GCN encoder (GCNConv -> LN -> GELU -> GCNConv -> LN).

Strategy (8 NeuronCores, SPMD), v2:
  - Nodes assigned to 784 global tiles of 128 (degree-balanced); core k owns
    tiles t with t%8==k (98 tiles = 12544 dst rows per core).
  - Layer 1: every core computes the FULL transformed table H1*dinv locally
    (X@W1 on all 784 tiles; no collective). Per-core table row order is a
    per-core permutation with the core's own tiles LAST so "my rows" sit at a
    core-independent offset.
  - Aggregation: normalization folded into the table (rows pre-scaled by
    dinv[src]) and the output (post-scaled by dinv[dst]); selector matrices
    are pure one-hot (single is_equal). Self-loops handled by adding the
    node's own table row (sequential read), not as gather edges.
  - Gathers: bf16 rows via dma_gather in fixed 2048-idx calls (16 blocks),
    fully padded (pad idx=0, pad slot=200 -> zero selector column), so no
    per-call count registers and no NaN-guard memsets.
  - Layer 2: transform locally (h1g @ W2, scaled by dinv), AllGather the
    bf16 table (core-major rows), aggregate the same way.
"""

from contextlib import ExitStack

import numpy as np

import concourse.bass as bass
import concourse.bacc as bacc
import concourse.mybir as mybir
import concourse.tile as tile
from concourse.bass_utils import run_bass_kernel_spmd

dt = mybir.dt
F32 = dt.float32
BF16 = dt.bfloat16

# -------- problem geometry (hardcoded for the graded problem) --------
N_FULL = 100000
IN_DIM = 256
HID2 = 256
HID = 128
N_CORES = 8
TILE = 128
NT = 784           # global tiles
TPC = 98           # tiles per core
SHARD = TPC * TILE # 12544
PADN = NT * TILE   # 100352
NCHUNK = 4
CH = PADN // NCHUNK  # 25088 (int16-safe)
CALLB = 16         # blocks per gather call (2048 idxs; multi-packet)
NI = CALLB * TILE  # 2048
MYBASE = (NT - TPC) * TILE  # 87808: per-core table rows of own tiles
PADSLOT = 200.0


# ============================ bass program builder ============================

def build_program(tc, io, geom):
    nc = tc.nc
    AOT = mybir.AluOpType
    AFT = mybir.ActivationFunctionType
    eps = 1e-5
    n_in_ch = IN_DIM // 128
    n_h_ch = HID2 // 128

    ctx = ExitStack()
    consts = ctx.enter_context(tc.tile_pool(name="consts", bufs=1))
    work = ctx.enter_context(tc.tile_pool(name="work", bufs=2))
    ln = ctx.enter_context(tc.tile_pool(name="ln", bufs=3))
    msgp = ctx.enter_context(tc.tile_pool(name="msgp", bufs=8))
    idxp = ctx.enter_context(tc.tile_pool(name="idxp", bufs=4))
    selp = ctx.enter_context(tc.tile_pool(name="selp", bufs=3))
    trowp = ctx.enter_context(tc.tile_pool(name="trowp", bufs=2))
    ps256 = ctx.enter_context(tc.tile_pool(name="ps256", bufs=2, space="PSUM"))
    ps128 = ctx.enter_context(tc.tile_pool(name="ps128", bufs=2, space="PSUM"))
    dram = ctx.enter_context(tc.tile_pool(name="dram", bufs=1, space="DRAM"))

    # ---- constants ----
    w1s = consts.tile([128, n_in_ch, HID2], BF16)
    nc.sync.dma_start(w1s[:], io["w1"].rearrange("(c p) n -> p c n", p=128))
    w2s = consts.tile([128, n_h_ch, HID], BF16)
    nc.sync.dma_start(w2s[:], io["w2"].rearrange("(c p) n -> p c n", p=128))
    bias1 = consts.tile([128, 3, HID2], F32)
    nc.sync.dma_start(bias1[:], io["bias1"])
    bias2 = consts.tile([128, 3, HID], F32)
    nc.sync.dma_start(bias2[:], io["bias2"])
    ident = consts.tile([128, 128], F32)
    nc.sync.dma_start(ident[:], io["ident"])
    iota_b = consts.tile([128, 128], BF16)
    nc.sync.dma_start(iota_b[:], io["iota_b"])
    dinv_t = consts.tile([128, NT], F32)
    nc.sync.dma_start(dinv_t[:], io["dinv_t"])
    dl1 = consts.tile([128, geom["NB1"]], BF16)
    nc.sync.dma_start(dl1[:], io["dl1"])
    dl2 = consts.tile([128, geom["NB2"]], BF16)
    nc.sync.dma_start(dl2[:], io["dl2"])
    eps_t = consts.tile([128, 1], F32)
    nc.vector.memset(eps_t[:], eps)
    c2048 = consts.tile([1, 1], dt.int32)
    nc.sync.dma_start(c2048[:], io["c2048"])
    r2048 = nc.alloc_register(mybir.EngineType.Pool, "gNI")
    nc.gpsimd.reg_load(r2048, c2048[:1, :1])

    # ---- DRAM buffers ----
    tab1c = [dram.tile([CH, HID2], BF16, name=f"tab1c{i}") for i in range(NCHUNK)]
    ag2_in = dram.tile([SHARD, HID], BF16)
    ag2_out = dram.tile([PADN, HID], BF16, addr_space="Shared")

    # ---- stage A: full local table1 = dinv * (X @ W1), bf16; 4 tiles/DMA ----
    for tb in range(0, NT, 4):
        xt_t = work.tile([128, n_in_ch, 4 * 128], BF16, tag="xt")
        nc.sync.dma_start(
            xt_t[:],
            io["xt"][:, tb * 128:(tb + 4) * 128].rearrange("(c p) n -> p c n", p=128))
        h1t = work.tile([128, 4, HID2], BF16, tag="h1t")
        for u in range(4):
            t = tb + u
            ps = ps256.tile([128, HID2], F32, tag="psA")
            for cc in range(n_in_ch):
                nc.tensor.matmul(ps[:], xt_t[:, cc, u * 128:(u + 1) * 128],
                                 w1s[:, cc, :],
                                 start=(cc == 0), stop=(cc == n_in_ch - 1))
            nc.scalar.activation(h1t[:, u, :], ps[:], AFT.Copy,
                                 scale=dinv_t[:, t:t + 1])
        tc_, tr = divmod(tb * 128, CH)
        nc.sync.dma_start(
            tab1c[tc_][tr:tr + 512, :].rearrange("(b p) f -> p b f", p=128),
            h1t[:])

    # ---- generic aggregation layer ----
    def agg_layer(tab_list, feat, B, S, CB, NC, dl_t, io_idx, bias_t, gelu, trow_src, out_cb):
        # emit all gather calls (Tile pipelines via pool WAR deps)
        msg_tiles = {}
        maxw = int(max(NC))
        for w in range(maxw):
            for cc in range(NCHUNK):
                if w >= NC[cc]:
                    continue
                it = idxp.tile([128, NI // 16], dt.int16, tag="idx")
                col0 = int(CB[cc] + w * CALLB) * 8
                nc.sync.dma_start(it[:], io_idx[:, col0:col0 + NI // 16])
                msg = msgp.tile([128, CALLB, feat], BF16, tag="msg")
                nc.gpsimd.dma_gather(
                    msg[:], tab_list[cc][:], it[:],
                    NI, r2048, feat, single_packet=False)
                msg_tiles[(cc, w)] = msg

        for lt in range(TPC):
            bt = int(B[lt].sum())
            assert bt > 0
            ps = ps256.tile([128, feat], F32, tag="psAgg")
            done = 0
            for cc in range(NCHUNK):
                bc = int(B[lt, cc])
                if bc == 0:
                    continue
                sel = selp.tile([128, bc, 128], BF16, tag="sel")
                g0 = int(CB[cc] + S[lt, cc])
                nc.vector.tensor_tensor(
                    sel[:],
                    iota_b[:].rearrange("p (b m) -> p b m", b=1).to_broadcast((128, bc, 128)),
                    dl_t[:, g0:g0 + bc].rearrange("p (b m) -> p b m", m=1).to_broadcast((128, bc, 128)),
                    AOT.is_equal)
                for bi in range(bc):
                    w, j = divmod(g0 + bi, CALLB)
                    w -= int(CB[cc]) // CALLB
                    msg = msg_tiles[(cc, w)]
                    nc.tensor.matmul(ps[:], sel[:, bi, :], msg[:, j, :],
                                     start=(done == 0), stop=(done == bt - 1))
                    done += 1
            # + own row (self loop), scale by dinv[dst], +bias, LN (+gelu)
            trow = trowp.tile([128, feat], BF16, tag="trow")
            nc.sync.dma_start(trow[:], trow_src[lt * 128:(lt + 1) * 128, :])
            t_mine = (NT - TPC) + lt  # position of my lt-th tile in per-core order
            xbA = ln.tile([128, feat], F32, tag="xbA")
            nc.vector.tensor_tensor(xbA[:], ps[:], trow[:], AOT.add)
            xb = ln.tile([128, feat], F32, tag="xb")
            r1 = ln.tile([128, 1], F32, tag="r1")
            nc.vector.scalar_tensor_tensor(xb[:], xbA[:], dinv_t[:, t_mine:t_mine + 1],
                                           bias_t[:, 0, :], AOT.mult, AOT.add,
                                           accum_out=r1[:])
            sq = ln.tile([128, feat], F32, tag="sq")
            r2 = ln.tile([128, 1], F32, tag="r2")
            nc.scalar.activation(sq[:], xb[:], AFT.Square, accum_out=r2[:])
            mu = ln.tile([128, 1], F32, tag="mu")
            nc.vector.tensor_scalar(mu[:], r1[:], 1.0 / feat, None, AOT.mult)
            musq = ln.tile([128, 1], F32, tag="musq")
            nc.vector.tensor_tensor(musq[:], mu[:], mu[:], AOT.mult)
            var = ln.tile([128, 1], F32, tag="var")
            nc.vector.tensor_scalar(var[:], r2[:], 1.0 / feat, musq[:],
                                    AOT.mult, AOT.subtract)
            st = ln.tile([128, 1], F32, tag="st")
            nc.scalar.activation(st[:], var[:], AFT.Sqrt, bias=eps_t[:])
            rstd = ln.tile([128, 1], F32, tag="rstd")
            nc.vector.reciprocal(rstd[:], st[:])
            xn = ln.tile([128, feat], F32, tag="xn")
            nc.vector.tensor_scalar(xn[:], xb[:], mu[:], rstd[:],
                                    AOT.subtract, AOT.mult)
            y = ln.tile([128, feat], F32, tag="y")
            nc.vector.tensor_tensor(y[:], xn[:], bias_t[:, 1, :], AOT.mult)
            nc.vector.tensor_tensor(y[:], y[:], bias_t[:, 2, :], AOT.add)
            if gelu:
                h = ln.tile([128, feat], F32, tag="h")
                nc.scalar.activation(h[:], y[:], AFT.Gelu)
                out_cb(lt, h)
            else:
                out_cb(lt, y)

    # ---- L1 -> transform to table2 rows (dinv * h1g @ W2) ----
    def l1_out(lt, h):
        t_mine = (NT - TPC) + lt
        h1T = work.tile([128, n_h_ch, 128], BF16, tag="h1T")
        for cc in range(n_h_ch):
            pst = ps128.tile([128, 128], F32, tag="psT")
            nc.tensor.transpose(pst[:], h[:, cc * 128:(cc + 1) * 128], ident[:])
            nc.vector.tensor_copy(h1T[:, cc, :], pst[:])
        ps2 = ps128.tile([128, HID], F32, tag="psC")
        for cc in range(n_h_ch):
            nc.tensor.matmul(ps2[:], h1T[:, cc, :], w2s[:, cc, :],
                             start=(cc == 0), stop=(cc == n_h_ch - 1))
        h2 = work.tile([128, HID], BF16, tag="h2")
        nc.scalar.activation(h2[:], ps2[:], AFT.Copy, scale=dinv_t[:, t_mine:t_mine + 1])
        nc.sync.dma_start(ag2_in[lt * 128:(lt + 1) * 128, :], h2[:])

    agg_layer(tab1c, HID2, geom["B1"], geom["S1"], geom["CB1"], geom["NC1"],
              dl1, io["idx1"], bias1, True, tab1c[3][MYBASE - 3 * CH:, :], l1_out)

    nc.gpsimd.collective_compute(
        "AllGather", AOT.bypass,
        replica_groups=[list(range(N_CORES))],
        ins=[ag2_in.opt()], outs=[ag2_out.opt()])

    # ---- L2 aggregation -> final output ----
    def l2_out(lt, y):
        o = work.tile([128, HID], F32, tag="o")
        nc.vector.tensor_copy(o[:], y[:])
        nc.sync.dma_start(io["out"][lt * 128:(lt + 1) * 128, :], o[:])

    agg_layer([ag2_out[cc * CH:(cc + 1) * CH, :] for cc in range(NCHUNK)],
              HID, geom["B2"], geom["S2"], geom["CB2"], geom["NC2"],
              dl2, io["idx2"], bias2, False, ag2_in[:], l2_out)
    ctx.close()


# ============================ top-level kernel ============================

def declare_io(nc, geom):
    io = {
        "xt": nc.dram_tensor("xt", [IN_DIM, PADN], BF16, kind="ExternalInput").ap(),
        "w1": nc.dram_tensor("w1", [IN_DIM, HID2], BF16, kind="ExternalInput").ap(),
        "w2": nc.dram_tensor("w2", [HID2, HID], BF16, kind="ExternalInput").ap(),
        "bias1": nc.dram_tensor("bias1", [128, 3, HID2], F32, kind="ExternalInput").ap(),
        "bias2": nc.dram_tensor("bias2", [128, 3, HID], F32, kind="ExternalInput").ap(),
        "iota_b": nc.dram_tensor("iota_b", [128, 128], BF16, kind="ExternalInput").ap(),
        "ident": nc.dram_tensor("ident", [128, 128], F32, kind="ExternalInput").ap(),
        "dinv_t": nc.dram_tensor("dinv_t", [128, NT], F32, kind="ExternalInput").ap(),
        "idx1": nc.dram_tensor("idx1", [128, geom["NB1"] * 8], dt.int16,
                               kind="ExternalInput").ap(),
        "dl1": nc.dram_tensor("dl1", [128, geom["NB1"]], BF16, kind="ExternalInput").ap(),
        "idx2": nc.dram_tensor("idx2", [128, geom["NB2"] * 8], dt.int16,
                               kind="ExternalInput").ap(),
        "dl2": nc.dram_tensor("dl2", [128, geom["NB2"]], BF16, kind="ExternalInput").ap(),
        "c2048": nc.dram_tensor("c2048", [1, 1], dt.int32, kind="ExternalInput").ap(),
        "out": nc.dram_tensor("out", [SHARD, HID], F32, kind="ExternalOutput").ap(),
    }
    return io


def kernel(x, edge_index, W1, b1, g1, be1, W2, b2, g2, be2,
           trace=False, _return_raw=False):
    bf = dt.np(BF16)
    x = np.asarray(x, np.float32)
    src = np.asarray(edge_index[0], np.int64)
    dst = np.asarray(edge_index[1], np.int64)
    N = x.shape[0]

    deg = (np.bincount(dst, minlength=N) + 1).astype(np.float32)
    dinv = (1.0 / np.sqrt(deg)).astype(np.float32)

    order = np.argsort(-deg, kind="stable")
    node_tile = np.empty(N, np.int32)
    node_slot = np.empty(N, np.int32)
    ar = np.arange(N, dtype=np.int64)
    node_tile[order] = (ar % NT).astype(np.int32)
    node_slot[order] = (ar // NT).astype(np.int32)
    core_of = node_tile % N_CORES
    lt_of = node_tile // N_CORES

    dinv_st = np.ones((TILE, NT), np.float32)
    dinv_st[node_slot, node_tile] = dinv
    row2 = core_of.astype(np.int64) * SHARD + lt_of.astype(np.int64) * TILE + node_slot

    # --- per-core packing ---
    cores = []
    cnts1, cnts2 = [], []
    for k in range(N_CORES):
        others = np.setdiff1d(np.arange(NT, dtype=np.int64),
                              np.arange(k, NT, N_CORES, dtype=np.int64),
                              assume_unique=True)
        mine = np.arange(k, NT, N_CORES, dtype=np.int64)
        tord = np.concatenate([others, mine])
        tpos = np.empty(NT, np.int64)
        tpos[tord] = np.arange(NT, dtype=np.int64)
        row1 = tpos[node_tile] * TILE + node_slot

        m = core_of[dst] == k
        elt = lt_of[dst[m]].astype(np.int64)
        eslot = node_slot[dst[m]].astype(np.float32)
        esrc = src[m]

        def sort_pack(srcrow):
            c = srcrow // CH
            i16 = (srcrow - c * CH).astype(np.int16)
            key = elt * NCHUNK + c
            o = np.argsort(key, kind="stable")
            cnts = np.bincount(key, minlength=TPC * NCHUNK).reshape(TPC, NCHUNK)
            return i16[o], eslot[o], cnts

        i16a, sla, ca = sort_pack(row1[esrc])
        i16b, slb, cb = sort_pack(row2[esrc])
        cnts1.append(ca)
        cnts2.append(cb)

        xs = np.zeros((PADN, IN_DIM), np.float32)
        xs[row1] = x
        cores.append(dict(
            xt=np.ascontiguousarray(xs.T).astype(bf),
            dinv_t=np.ascontiguousarray(dinv_st[:, tord]),
            e1=(i16a, sla, ca), e2=(i16b, slb, cb),
            nodes=np.nonzero(core_of == k)[0]))

    B1, S1, CB1, NC1, NB1 = finalize_geometry(cnts1)
    B2, S2, CB2, NC2, NB2 = finalize_geometry(cnts2)
    geom = dict(B1=B1, S1=S1, CB1=CB1, NC1=NC1, NB1=NB1,
                B2=B2, S2=S2, CB2=CB2, NC2=NC2, NB2=NB2)

    iota_np = np.tile(np.arange(128, dtype=np.float32)[None, :], (128, 1)).astype(bf)
    ident_np = np.eye(128, dtype=np.float32)
    bias1_np = np.broadcast_to(
        np.stack([np.asarray(b1, np.float32), np.asarray(g1, np.float32),
                  np.asarray(be1, np.float32)])[None], (128, 3, HID2)).copy()
    bias2_np = np.broadcast_to(
        np.stack([np.asarray(b2, np.float32), np.asarray(g2, np.float32),
                  np.asarray(be2, np.float32)])[None], (128, 3, HID)).copy()

    in_maps = []
    for k in range(N_CORES):
        pc = cores[k]
        idx1, dl1 = build_core_arrays(pc["e1"], B1, S1, CB1, NB1)
        idx2, dl2 = build_core_arrays(pc["e2"], B2, S2, CB2, NB2)
        in_maps.append({
            "xt": pc["xt"], "w1": np.asarray(W1, np.float32).astype(bf),
            "w2": np.asarray(W2, np.float32).astype(bf),
            "bias1": bias1_np, "bias2": bias2_np,
            "iota_b": iota_np, "ident": ident_np,
            "dinv_t": pc["dinv_t"],
            "idx1": idx1, "dl1": dl1, "idx2": idx2, "dl2": dl2,
            "c2048": np.array([[NI]], np.int32),
        })

    nc = bacc.Bacc("TRN2", debug=False, num_devices=N_CORES)
    io = declare_io(nc, geom)
    with tile.TileContext(nc) as tc:
        build_program(tc, io, geom)
    nc.compile()

    res = run_bass_kernel_spmd(nc, in_maps, core_ids=list(range(N_CORES)),
                               trace=trace)
    out = np.empty((N, HID), np.float32)
    for k in range(N_CORES):
        pc = cores[k]
        ok = np.asarray(res.results[k]["out"])
        pos = lt_of[pc["nodes"]] * TILE + node_slot[pc["nodes"]]
        out[pc["nodes"]] = ok[pos]
    if _return_raw:
        return out, res
    return out


def build_core_arrays(epack, B, S, CB, NB):
    bf = dt.np(BF16)
    i16, slot, cnts = epack
    idx_a = np.zeros((16, NB * 8), np.int16)
    dl_a = np.full((TILE, NB), PADSLOT, np.float32)
    starts = np.zeros(TPC * NCHUNK + 1, np.int64)
    np.cumsum(cnts.reshape(-1), out=starts[1:])
    for lt in range(TPC):
        for cc in range(NCHUNK):
            m = int(cnts[lt, cc])
            if m == 0:
                continue
            s0 = int(starts[lt * NCHUNK + cc])
            p = (int(CB[cc] + S[lt, cc])) * TILE + np.arange(m)
            idx_a[p % 16, p // 16] = i16[s0:s0 + m]
            dl_a[p % TILE, p // TILE] = slot[s0:s0 + m]
    return np.tile(idx_a, (8, 1)), dl_a.astype(bf)


def finalize_geometry(cnts_list):
    allc = np.stack(cnts_list)  # [8, TPC, NCHUNK]
    B = (-(-allc.max(axis=0) // TILE)).astype(np.int64)
    S = np.zeros((TPC, NCHUNK), np.int64)
    CB = np.zeros(NCHUNK + 1, np.int64)
    NC = np.zeros(NCHUNK, np.int64)
    for cc in range(NCHUNK):
        S[:, cc] = np.cumsum(B[:, cc]) - B[:, cc]
        nb = int(B[:, cc].sum())
        NC[cc] = -(-nb // CALLB)
        CB[cc + 1] = CB[cc] + NC[cc] * CALLB
    return B, S, CB, NC, int(CB[NCHUNK])


# revision 11
# speedup vs baseline: 1.3535x; 1.0206x over previous
"""Trainium2 Bass kernel for a 2-layer GCN encoder (GCNConv -> LN -> GELU -> GCNConv -> LN).

Strategy (8 NeuronCores, SPMD), v2:
  - Nodes assigned to 784 global tiles of 128 (degree-balanced); core k owns
    tiles t with t%8==k (98 tiles = 12544 dst rows per core).
  - Layer 1: every core computes the FULL transformed table H1*dinv locally
    (X@W1 on all 784 tiles; no collective). Per-core table row order is a
    per-core permutation with the core's own tiles LAST so "my rows" sit at a
    core-independent offset.
  - Aggregation: normalization folded into the table (rows pre-scaled by
    dinv[src]) and the output (post-scaled by dinv[dst]); selector matrices
    are pure one-hot (single is_equal). Self-loops handled by adding the
    node's own table row (sequential read), not as gather edges.
  - Gathers: bf16 rows via dma_gather in fixed 2048-idx calls (16 blocks),
    fully padded (pad idx=0, pad slot=200 -> zero selector column), so no
    per-call count registers and no NaN-guard memsets.
  - Layer 2: transform locally (h1g @ W2, scaled by dinv), AllGather the
    bf16 table (core-major rows), aggregate the same way.
"""

from contextlib import ExitStack

import numpy as np

import concourse.bass as bass
import concourse.bacc as bacc
import concourse.mybir as mybir
import concourse.tile as tile
from concourse.bass_utils import run_bass_kernel_spmd

dt = mybir.dt
F32 = dt.float32
BF16 = dt.bfloat16

# -------- problem geometry (hardcoded for the graded problem) --------
N_FULL = 100000
IN_DIM = 256
HID2 = 256
HID = 128
N_CORES = 8
TILE = 128
NT = 784           # global tiles
TPC = 98           # tiles per core
SHARD = TPC * TILE # 12544
PADN = NT * TILE   # 100352
NCHUNK = 4
CH = PADN // NCHUNK  # 25088 (int16-safe)
CALLB = 24         # blocks per gather call (3072 idxs; multi-packet)
NI = CALLB * TILE  # 2048
MYBASE = (NT - TPC) * TILE  # 87808: per-core table rows of own tiles
PADSLOT = 200.0


# ============================ bass program builder ============================

def build_program(tc, io, geom):
    nc = tc.nc
    AOT = mybir.AluOpType
    AFT = mybir.ActivationFunctionType
    eps = 1e-5
    n_in_ch = IN_DIM // 128
    n_h_ch = HID2 // 128

    ctx = ExitStack()
    consts = ctx.enter_context(tc.tile_pool(name="consts", bufs=1))
    work = ctx.enter_context(tc.tile_pool(name="work", bufs=2))
    ln = ctx.enter_context(tc.tile_pool(name="ln", bufs=3))
    msgp = ctx.enter_context(tc.tile_pool(name="msgp", bufs=7))
    idxp = ctx.enter_context(tc.tile_pool(name="idxp", bufs=4))
    selp = ctx.enter_context(tc.tile_pool(name="selp", bufs=3))
    trowp = ctx.enter_context(tc.tile_pool(name="trowp", bufs=2))
    ps256 = ctx.enter_context(tc.tile_pool(name="ps256", bufs=2, space="PSUM"))
    ps128 = ctx.enter_context(tc.tile_pool(name="ps128", bufs=2, space="PSUM"))
    dram = ctx.enter_context(tc.tile_pool(name="dram", bufs=1, space="DRAM"))

    # ---- constants ----
    w1s = consts.tile([128, n_in_ch, HID2], BF16)
    nc.sync.dma_start(w1s[:], io["w1"].rearrange("(c p) n -> p c n", p=128))
    w2s = consts.tile([128, n_h_ch, HID], BF16)
    nc.sync.dma_start(w2s[:], io["w2"].rearrange("(c p) n -> p c n", p=128))
    bias1 = consts.tile([128, 3, HID2], F32)
    nc.sync.dma_start(bias1[:], io["bias1"])
    bias2 = consts.tile([128, 3, HID], F32)
    nc.sync.dma_start(bias2[:], io["bias2"])
    ident = consts.tile([128, 128], F32)
    nc.sync.dma_start(ident[:], io["ident"])
    iota_b = consts.tile([128, 128], BF16)
    nc.sync.dma_start(iota_b[:], io["iota_b"])
    dinv_t = consts.tile([128, NT], F32)
    nc.sync.dma_start(dinv_t[:], io["dinv_t"])
    dl1 = consts.tile([128, geom["NB1"]], BF16)
    nc.sync.dma_start(dl1[:], io["dl1"])
    dl2 = consts.tile([128, geom["NB2"]], BF16)
    nc.sync.dma_start(dl2[:], io["dl2"])
    eps_t = consts.tile([128, 1], F32)
    nc.vector.memset(eps_t[:], eps)
    c2048 = consts.tile([1, 1], dt.int32)
    nc.sync.dma_start(c2048[:], io["c2048"])
    r2048 = nc.alloc_register(mybir.EngineType.Pool, "gNI")
    nc.gpsimd.reg_load(r2048, c2048[:1, :1])

    # ---- DRAM buffers ----
    tab1c = [dram.tile([CH, HID2], BF16, name=f"tab1c{i}") for i in range(NCHUNK)]
    ag2_in = dram.tile([SHARD, HID], BF16)
    ag2_out = dram.tile([PADN, HID], BF16, addr_space="Shared")

    # ---- stage A: full local table1 = dinv * (X @ W1), bf16; 4 tiles/DMA ----
    for tb in range(0, NT, 14):
        xt_t = work.tile([128, n_in_ch, 14 * 128], BF16, tag="xt")
        nc.sync.dma_start(
            xt_t[:],
            io["xt"][:, tb * 128:(tb + 14) * 128].rearrange("(c p) n -> p c n", p=128))
        h1t = work.tile([128, 14, HID2], BF16, tag="h1t")
        for u in range(14):
            t = tb + u
            ps = ps256.tile([128, HID2], F32, tag="psA")
            for cc in range(n_in_ch):
                nc.tensor.matmul(ps[:], xt_t[:, cc, u * 128:(u + 1) * 128],
                                 w1s[:, cc, :],
                                 start=(cc == 0), stop=(cc == n_in_ch - 1))
            nc.scalar.activation(h1t[:, u, :], ps[:], AFT.Copy,
                                 scale=dinv_t[:, t:t + 1])
        tc_, tr = divmod(tb * 128, CH)
        nc.sync.dma_start(
            tab1c[tc_][tr:tr + 14 * 128, :].rearrange("(b p) f -> p b f", p=128),
            h1t[:])

    # ---- generic aggregation layer ----
    def agg_layer(tab_list, feat, B, S, CB, NC, dl_t, io_idx, bias_t, gelu, trow_src, out_cb):
        # emit all gather calls (Tile pipelines via pool WAR deps)
        msg_tiles = {}
        maxw = int(max(NC))
        for w in range(maxw):
            for cc in range(NCHUNK):
                if w >= NC[cc]:
                    continue
                it = idxp.tile([128, NI // 16], dt.int16, tag="idx")
                col0 = int(CB[cc] + w * CALLB) * 8
                nc.sync.dma_start(it[:], io_idx[:, col0:col0 + NI // 16])
                msg = msgp.tile([128, CALLB, feat], BF16, tag="msg")
                nc.gpsimd.dma_gather(
                    msg[:], tab_list[cc][:], it[:],
                    NI, r2048, feat, single_packet=False)
                msg_tiles[(cc, w)] = msg

        for lt in range(TPC):
            bt = int(B[lt].sum())
            assert bt > 0
            ps = ps256.tile([128, feat], F32, tag="psAgg")
            done = 0
            for cc in range(NCHUNK):
                bc = int(B[lt, cc])
                if bc == 0:
                    continue
                sel = selp.tile([128, bc, 128], BF16, tag="sel")
                g0 = int(CB[cc] + S[lt, cc])
                nc.vector.tensor_tensor(
                    sel[:],
                    iota_b[:].rearrange("p (b m) -> p b m", b=1).to_broadcast((128, bc, 128)),
                    dl_t[:, g0:g0 + bc].rearrange("p (b m) -> p b m", m=1).to_broadcast((128, bc, 128)),
                    AOT.is_equal)
                for bi in range(bc):
                    w, j = divmod(g0 + bi, CALLB)
                    w -= int(CB[cc]) // CALLB
                    msg = msg_tiles[(cc, w)]
                    nc.tensor.matmul(ps[:], sel[:, bi, :], msg[:, j, :],
                                     start=(done == 0), stop=(done == bt - 1))
                    done += 1
            # + own row (self loop), scale by dinv[dst], +bias, LN (+gelu)
            trow = trowp.tile([128, feat], BF16, tag="trow")
            nc.sync.dma_start(trow[:], trow_src[lt * 128:(lt + 1) * 128, :])
            t_mine = (NT - TPC) + lt  # position of my lt-th tile in per-core order
            xbA = ln.tile([128, feat], F32, tag="xbA")
            nc.vector.tensor_tensor(xbA[:], ps[:], trow[:], AOT.add)
            xb = ln.tile([128, feat], F32, tag="xb")
            r1 = ln.tile([128, 1], F32, tag="r1")
            nc.vector.scalar_tensor_tensor(xb[:], xbA[:], dinv_t[:, t_mine:t_mine + 1],
                                           bias_t[:, 0, :], AOT.mult, AOT.add,
                                           accum_out=r1[:])
            sq = ln.tile([128, feat], F32, tag="sq")
            r2 = ln.tile([128, 1], F32, tag="r2")
            nc.scalar.activation(sq[:], xb[:], AFT.Square, accum_out=r2[:])
            mu = ln.tile([128, 1], F32, tag="mu")
            nc.vector.tensor_scalar(mu[:], r1[:], 1.0 / feat, None, AOT.mult)
            musq = ln.tile([128, 1], F32, tag="musq")
            nc.vector.tensor_tensor(musq[:], mu[:], mu[:], AOT.mult)
            var = ln.tile([128, 1], F32, tag="var")
            nc.vector.tensor_scalar(var[:], r2[:], 1.0 / feat, musq[:],
                                    AOT.mult, AOT.subtract)
            st = ln.tile([128, 1], F32, tag="st")
            nc.scalar.activation(st[:], var[:], AFT.Sqrt, bias=eps_t[:])
            rstd = ln.tile([128, 1], F32, tag="rstd")
            nc.vector.reciprocal(rstd[:], st[:])
            xn = ln.tile([128, feat], F32, tag="xn")
            nc.vector.tensor_scalar(xn[:], xb[:], mu[:], rstd[:],
                                    AOT.subtract, AOT.mult)
            y = ln.tile([128, feat], F32, tag="y")
            nc.vector.tensor_tensor(y[:], xn[:], bias_t[:, 1, :], AOT.mult)
            nc.vector.tensor_tensor(y[:], y[:], bias_t[:, 2, :], AOT.add)
            if gelu:
                h = ln.tile([128, feat], F32, tag="h")
                nc.scalar.activation(h[:], y[:], AFT.Gelu)
                out_cb(lt, h)
            else:
                out_cb(lt, y)

    # ---- L1 -> transform to table2 rows (dinv * h1g @ W2) ----
    def l1_out(lt, h):
        t_mine = (NT - TPC) + lt
        h1T = work.tile([128, n_h_ch, 128], BF16, tag="h1T")
        for cc in range(n_h_ch):
            pst = ps128.tile([128, 128], F32, tag="psT")
            nc.tensor.transpose(pst[:], h[:, cc * 128:(cc + 1) * 128], ident[:])
            nc.vector.tensor_copy(h1T[:, cc, :], pst[:])
        ps2 = ps128.tile([128, HID], F32, tag="psC")
        for cc in range(n_h_ch):
            nc.tensor.matmul(ps2[:], h1T[:, cc, :], w2s[:, cc, :],
                             start=(cc == 0), stop=(cc == n_h_ch - 1))
        h2 = work.tile([128, HID], BF16, tag="h2")
        nc.scalar.activation(h2[:], ps2[:], AFT.Copy, scale=dinv_t[:, t_mine:t_mine + 1])
        nc.sync.dma_start(ag2_in[lt * 128:(lt + 1) * 128, :], h2[:])

    agg_layer(tab1c, HID2, geom["B1"], geom["S1"], geom["CB1"], geom["NC1"],
              dl1, io["idx1"], bias1, True, tab1c[3][MYBASE - 3 * CH:, :], l1_out)

    nc.gpsimd.collective_compute(
        "AllGather", AOT.bypass,
        replica_groups=[list(range(N_CORES))],
        ins=[ag2_in.opt()], outs=[ag2_out.opt()])

    # ---- L2 aggregation -> final output ----
    def l2_out(lt, y):
        o = work.tile([128, HID], F32, tag="o")
        nc.vector.tensor_copy(o[:], y[:])
        nc.sync.dma_start(io["out"][lt * 128:(lt + 1) * 128, :], o[:])

    agg_layer([ag2_out[cc * CH:(cc + 1) * CH, :] for cc in range(NCHUNK)],
              HID, geom["B2"], geom["S2"], geom["CB2"], geom["NC2"],
              dl2, io["idx2"], bias2, False, ag2_in[:], l2_out)
    ctx.close()


# ============================ top-level kernel ============================

def declare_io(nc, geom):
    io = {
        "xt": nc.dram_tensor("xt", [IN_DIM, PADN], BF16, kind="ExternalInput").ap(),
        "w1": nc.dram_tensor("w1", [IN_DIM, HID2], BF16, kind="ExternalInput").ap(),
        "w2": nc.dram_tensor("w2", [HID2, HID], BF16, kind="ExternalInput").ap(),
        "bias1": nc.dram_tensor("bias1", [128, 3, HID2], F32, kind="ExternalInput").ap(),
        "bias2": nc.dram_tensor("bias2", [128, 3, HID], F32, kind="ExternalInput").ap(),
        "iota_b": nc.dram_tensor("iota_b", [128, 128], BF16, kind="ExternalInput").ap(),
        "ident": nc.dram_tensor("ident", [128, 128], F32, kind="ExternalInput").ap(),
        "dinv_t": nc.dram_tensor("dinv_t", [128, NT], F32, kind="ExternalInput").ap(),
        "idx1": nc.dram_tensor("idx1", [128, geom["NB1"] * 8], dt.int16,
                               kind="ExternalInput").ap(),
        "dl1": nc.dram_tensor("dl1", [128, geom["NB1"]], BF16, kind="ExternalInput").ap(),
        "idx2": nc.dram_tensor("idx2", [128, geom["NB2"] * 8], dt.int16,
                               kind="ExternalInput").ap(),
        "dl2": nc.dram_tensor("dl2", [128, geom["NB2"]], BF16, kind="ExternalInput").ap(),
        "c2048": nc.dram_tensor("c2048", [1, 1], dt.int32, kind="ExternalInput").ap(),
        "out": nc.dram_tensor("out", [SHARD, HID], F32, kind="ExternalOutput").ap(),
    }
    return io


def kernel(x, edge_index, W1, b1, g1, be1, W2, b2, g2, be2,
           trace=False, _return_raw=False):
    bf = dt.np(BF16)
    x = np.asarray(x, np.float32)
    src = np.asarray(edge_index[0], np.int64)
    dst = np.asarray(edge_index[1], np.int64)
    N = x.shape[0]

    deg = (np.bincount(dst, minlength=N) + 1).astype(np.float32)
    dinv = (1.0 / np.sqrt(deg)).astype(np.float32)

    order = np.argsort(-deg, kind="stable")
    node_tile = np.empty(N, np.int32)
    node_slot = np.empty(N, np.int32)
    ar = np.arange(N, dtype=np.int64)
    node_tile[order] = (ar % NT).astype(np.int32)
    node_slot[order] = (ar // NT).astype(np.int32)
    core_of = node_tile % N_CORES
    lt_of = node_tile // N_CORES

    dinv_st = np.ones((TILE, NT), np.float32)
    dinv_st[node_slot, node_tile] = dinv
    row2 = core_of.astype(np.int64) * SHARD + lt_of.astype(np.int64) * TILE + node_slot

    # --- per-core packing ---
    cores = []
    cnts1, cnts2 = [], []
    for k in range(N_CORES):
        others = np.setdiff1d(np.arange(NT, dtype=np.int64),
                              np.arange(k, NT, N_CORES, dtype=np.int64),
                              assume_unique=True)
        mine = np.arange(k, NT, N_CORES, dtype=np.int64)
        tord = np.concatenate([others, mine])
        tpos = np.empty(NT, np.int64)
        tpos[tord] = np.arange(NT, dtype=np.int64)
        row1 = tpos[node_tile] * TILE + node_slot

        m = core_of[dst] == k
        elt = lt_of[dst[m]].astype(np.int64)
        eslot = node_slot[dst[m]].astype(np.float32)
        esrc = src[m]

        def sort_pack(srcrow):
            c = srcrow // CH
            i16 = (srcrow - c * CH).astype(np.int16)
            key = elt * NCHUNK + c
            o = np.argsort(key, kind="stable")
            cnts = np.bincount(key, minlength=TPC * NCHUNK).reshape(TPC, NCHUNK)
            return i16[o], eslot[o], cnts

        i16a, sla, ca = sort_pack(row1[esrc])
        i16b, slb, cb = sort_pack(row2[esrc])
        cnts1.append(ca)
        cnts2.append(cb)

        xs = np.zeros((PADN, IN_DIM), np.float32)
        xs[row1] = x
        cores.append(dict(
            xt=np.ascontiguousarray(xs.T).astype(bf),
            dinv_t=np.ascontiguousarray(dinv_st[:, tord]),
            e1=(i16a, sla, ca), e2=(i16b, slb, cb),
            nodes=np.nonzero(core_of == k)[0]))

    B1, S1, CB1, NC1, NB1 = finalize_geometry(cnts1)
    B2, S2, CB2, NC2, NB2 = finalize_geometry(cnts2)
    geom = dict(B1=B1, S1=S1, CB1=CB1, NC1=NC1, NB1=NB1,
                B2=B2, S2=S2, CB2=CB2, NC2=NC2, NB2=NB2)

    iota_np = np.tile(np.arange(128, dtype=np.float32)[None, :], (128, 1)).astype(bf)
    ident_np = np.eye(128, dtype=np.float32)
    bias1_np = np.broadcast_to(
        np.stack([np.asarray(b1, np.float32), np.asarray(g1, np.float32),
                  np.asarray(be1, np.float32)])[None], (128, 3, HID2)).copy()
    bias2_np = np.broadcast_to(
        np.stack([np.asarray(b2, np.float32), np.asarray(g2, np.float32),
                  np.asarray(be2, np.float32)])[None], (128, 3, HID)).copy()

    in_maps = []
    for k in range(N_CORES):
        pc = cores[k]
        idx1, dl1 = build_core_arrays(pc["e1"], B1, S1, CB1, NB1)
        idx2, dl2 = build_core_arrays(pc["e2"], B2, S2, CB2, NB2)
        in_maps.append({
            "xt": pc["xt"], "w1": np.asarray(W1, np.float32).astype(bf),
            "w2": np.asarray(W2, np.float32).astype(bf),
            "bias1": bias1_np, "bias2": bias2_np,
            "iota_b": iota_np, "ident": ident_np,
            "dinv_t": pc["dinv_t"],
            "idx1": idx1, "dl1": dl1, "idx2": idx2, "dl2": dl2,
            "c2048": np.array([[NI]], np.int32),
        })

    nc = bacc.Bacc("TRN2", debug=False, num_devices=N_CORES)
    io = declare_io(nc, geom)
    with tile.TileContext(nc) as tc:
        build_program(tc, io, geom)
    nc.compile()

    res = run_bass_kernel_spmd(nc, in_maps, core_ids=list(range(N_CORES)),
                               trace=trace)
    out = np.empty((N, HID), np.float32)
    for k in range(N_CORES):
        pc = cores[k]
        ok = np.asarray(res.results[k]["out"])
        pos = lt_of[pc["nodes"]] * TILE + node_slot[pc["nodes"]]
        out[pc["nodes"]] = ok[pos]
    if _return_raw:
        return out, res
    return out


def build_core_arrays(epack, B, S, CB, NB):
    bf = dt.np(BF16)
    i16, slot, cnts = epack
    idx_a = np.zeros((16, NB * 8), np.int16)
    dl_a = np.full((TILE, NB), PADSLOT, np.float32)
    starts = np.zeros(TPC * NCHUNK + 1, np.int64)
    np.cumsum(cnts.reshape(-1), out=starts[1:])
    for lt in range(TPC):
        for cc in range(NCHUNK):
            m = int(cnts[lt, cc])
            if m == 0:
                continue
            s0 = int(starts[lt * NCHUNK + cc])
            p = (int(CB[cc] + S[lt, cc])) * TILE + np.arange(m)
            idx_a[p % 16, p // 16] = i16[s0:s0 + m]
            dl_a[p % TILE, p // TILE] = slot[s0:s0 + m]
    return np.tile(idx_a, (8, 1)), dl_a.astype(bf)


def finalize_geometry(cnts_list):
    allc = np.stack(cnts_list)  # [8, TPC, NCHUNK]
    B = (-(-allc.max(axis=0) // TILE)).astype(np.int64)
    S = np.zeros((TPC, NCHUNK), np.int64)
    CB = np.zeros(NCHUNK + 1, np.int64)
    NC = np.zeros(NCHUNK, np.int64)
    for cc in range(NCHUNK):
        S[:, cc] = np.cumsum(B[:, cc]) - B[:, cc]
        nb = int(B[:, cc].sum())
        NC[cc] = -(-nb // CALLB)
        CB[cc + 1] = CB[cc] + NC[cc] * CALLB
    return B, S, CB, NC, int(CB[NCHUNK])


# revision 13
# speedup vs baseline: 1.3805x; 1.0199x over previous
"""Trainium2 Bass kernel for a 2-layer GCN encoder (GCNConv -> LN -> GELU -> GCNConv -> LN).

Strategy (8 NeuronCores, SPMD), v2:
  - Nodes assigned to 784 global tiles of 128 (degree-balanced); core k owns
    tiles t with t%8==k (98 tiles = 12544 dst rows per core).
  - Layer 1: every core computes the FULL transformed table H1*dinv locally
    (X@W1 on all 784 tiles; no collective). Per-core table row order is a
    per-core permutation with the core's own tiles LAST so "my rows" sit at a
    core-independent offset.
  - Aggregation: normalization folded into the table (rows pre-scaled by
    dinv[src]) and the output (post-scaled by dinv[dst]); selector matrices
    are pure one-hot (single is_equal). Self-loops handled by adding the
    node's own table row (sequential read), not as gather edges.
  - Gathers: bf16 rows via dma_gather in fixed 3072-idx multi-packet calls,
    fully padded (pad idx=0, pad slot=200 -> zero selector column), so no
    per-call count registers and no NaN-guard memsets.
  - Layer 2: transform locally (h1g @ W2, scaled by dinv), AllGather the
    bf16 table (core-major rows), aggregate the same way.
"""

from contextlib import ExitStack

import numpy as np

import concourse.bass as bass
import concourse.bacc as bacc
import concourse.mybir as mybir
import concourse.tile as tile
from concourse.bass_utils import run_bass_kernel_spmd

dt = mybir.dt
F32 = dt.float32
BF16 = dt.bfloat16

# -------- problem geometry (hardcoded for the graded problem) --------
N_FULL = 100000
IN_DIM = 256
HID2 = 256
HID = 128
N_CORES = 8
TILE = 128
NT = 784           # global tiles
TPC = 98           # tiles per core
SHARD = TPC * TILE # 12544
PADN = NT * TILE   # 100352
NCHUNK = 4
CH = PADN // NCHUNK  # 25088 (int16-safe)
CALLB = 24         # blocks per gather call (3072 idxs; multi-packet)
NI = CALLB * TILE  # 3072
MYBASE = (NT - TPC) * TILE  # 87808: per-core table rows of own tiles
PADSLOT = 200.0


# ============================ bass program builder ============================

def build_program(tc, io, geom):
    nc = tc.nc
    AOT = mybir.AluOpType
    AFT = mybir.ActivationFunctionType
    eps = 1e-5
    n_in_ch = IN_DIM // 128
    n_h_ch = HID2 // 128

    ctx = ExitStack()
    consts = ctx.enter_context(tc.tile_pool(name="consts", bufs=1))
    work = ctx.enter_context(tc.tile_pool(name="work", bufs=2))
    ln = ctx.enter_context(tc.tile_pool(name="ln", bufs=3))
    msgp = ctx.enter_context(tc.tile_pool(name="msgp", bufs=8))
    idxp = ctx.enter_context(tc.tile_pool(name="idxp", bufs=4))
    selp = ctx.enter_context(tc.tile_pool(name="selp", bufs=3))
    trowp = ctx.enter_context(tc.tile_pool(name="trowp", bufs=2))
    ps256 = ctx.enter_context(tc.tile_pool(name="ps256", bufs=2, space="PSUM"))
    ps128 = ctx.enter_context(tc.tile_pool(name="ps128", bufs=2, space="PSUM"))
    dram = ctx.enter_context(tc.tile_pool(name="dram", bufs=1, space="DRAM"))

    # ---- constants ----
    w1s = consts.tile([128, n_in_ch, HID2], BF16)
    nc.sync.dma_start(w1s[:], io["w1"].rearrange("(c p) n -> p c n", p=128))
    w2s = consts.tile([128, n_h_ch, HID], BF16)
    nc.sync.dma_start(w2s[:], io["w2"].rearrange("(c p) n -> p c n", p=128))
    bias1 = consts.tile([128, 3, HID2], F32)
    nc.sync.dma_start(bias1[:], io["bias1"])
    bias2 = consts.tile([128, 3, HID], F32)
    nc.sync.dma_start(bias2[:], io["bias2"])
    ident = consts.tile([128, 128], F32)
    nc.sync.dma_start(ident[:], io["ident"])
    iota_b = consts.tile([128, 128], BF16)
    nc.sync.dma_start(iota_b[:], io["iota_b"])
    dinv_t = consts.tile([128, NT], F32)
    nc.sync.dma_start(dinv_t[:], io["dinv_t"])
    dl1 = consts.tile([128, geom["NB1"]], BF16)
    nc.sync.dma_start(dl1[:], io["dl1"])
    dl2 = consts.tile([128, geom["NB2"]], BF16)
    nc.sync.dma_start(dl2[:], io["dl2"])
    eps_t = consts.tile([128, 1], F32)
    nc.vector.memset(eps_t[:], eps)
    c2048 = consts.tile([1, 1], dt.int32)
    nc.sync.dma_start(c2048[:], io["c2048"])
    r2048 = nc.alloc_register(mybir.EngineType.Pool, "gNI")
    nc.gpsimd.reg_load(r2048, c2048[:1, :1])

    # ---- DRAM buffers ----
    tab1c = [dram.tile([CH, HID2], BF16, name=f"tab1c{i}") for i in range(NCHUNK)]
    ag2_in = dram.tile([SHARD, HID], BF16)
    ag2_out = dram.tile([PADN, HID], BF16, addr_space="Shared")

    # ---- stage A: full local table1 = dinv * (X @ W1), bf16; 4 tiles/DMA ----
    for tb in range(0, NT, 14):
        xt_t = work.tile([128, n_in_ch, 14 * 128], BF16, tag="xt")
        nc.sync.dma_start(
            xt_t[:],
            io["xt"][:, tb * 128:(tb + 14) * 128].rearrange("(c p) n -> p c n", p=128))
        h1t = work.tile([128, 14, HID2], BF16, tag="h1t")
        for u in range(14):
            t = tb + u
            ps = ps256.tile([128, HID2], F32, tag="psA")
            for cc in range(n_in_ch):
                nc.tensor.matmul(ps[:], xt_t[:, cc, u * 128:(u + 1) * 128],
                                 w1s[:, cc, :],
                                 start=(cc == 0), stop=(cc == n_in_ch - 1))
            nc.scalar.activation(h1t[:, u, :], ps[:], AFT.Copy,
                                 scale=dinv_t[:, t:t + 1])
        tc_, tr = divmod(tb * 128, CH)
        nc.sync.dma_start(
            tab1c[tc_][tr:tr + 14 * 128, :].rearrange("(b p) f -> p b f", p=128),
            h1t[:])

    # ---- generic aggregation layer ----
    def agg_layer(tab_list, feat, B, S, CB, NC, dl_t, io_idx, bias_t, gelu, trow_src, out_cb):
        # emit all gather calls (Tile pipelines via pool WAR deps).
        # Warmup: two windows per chunk in chunk-completion order, so the Q7
        # gathers from early chunks while stage A still builds later chunks.
        msg_tiles = {}
        maxw = int(max(NC))
        WARM = 2
        emit_order = [(cc, w) for cc in range(NCHUNK)
                      for w in range(min(WARM, int(NC[cc])))]
        emit_order += [(cc, w) for w in range(WARM, maxw)
                       for cc in range(NCHUNK) if w < NC[cc]]
        for cc, w in emit_order:
            if True:
                it = idxp.tile([128, NI // 16], dt.int16, tag="idx")
                col0 = int(CB[cc] + w * CALLB) * 8
                nc.sync.dma_start(it[:], io_idx[:, col0:col0 + NI // 16])
                msg = msgp.tile([128, CALLB, feat], BF16, tag="msg")
                nc.gpsimd.dma_gather(
                    msg[:], tab_list[cc][:], it[:],
                    NI, r2048, feat, single_packet=False)
                msg_tiles[(cc, w)] = msg

        for lt in range(TPC):
            bt = int(B[lt].sum())
            assert bt > 0
            ps = ps256.tile([128, feat], F32, tag="psAgg")
            done = 0
            for cc in range(NCHUNK):
                bc = int(B[lt, cc])
                if bc == 0:
                    continue
                sel = selp.tile([128, bc, 128], BF16, tag="sel")
                g0 = int(CB[cc] + S[lt, cc])
                nc.vector.tensor_tensor(
                    sel[:],
                    iota_b[:].rearrange("p (b m) -> p b m", b=1).to_broadcast((128, bc, 128)),
                    dl_t[:, g0:g0 + bc].rearrange("p (b m) -> p b m", m=1).to_broadcast((128, bc, 128)),
                    AOT.is_equal)
                for bi in range(bc):
                    w, j = divmod(g0 + bi, CALLB)
                    w -= int(CB[cc]) // CALLB
                    msg = msg_tiles[(cc, w)]
                    nc.tensor.matmul(ps[:], sel[:, bi, :], msg[:, j, :],
                                     start=(done == 0), stop=(done == bt - 1))
                    done += 1
            # + own row (self loop), scale by dinv[dst], +bias, LN (+gelu)
            trow = trowp.tile([128, feat], BF16, tag="trow")
            nc.sync.dma_start(trow[:], trow_src[lt * 128:(lt + 1) * 128, :])
            t_mine = (NT - TPC) + lt  # position of my lt-th tile in per-core order
            xbA = ln.tile([128, feat], F32, tag="xbA")
            nc.vector.tensor_tensor(xbA[:], ps[:], trow[:], AOT.add)
            xb = ln.tile([128, feat], F32, tag="xb")
            r1 = ln.tile([128, 1], F32, tag="r1")
            nc.vector.scalar_tensor_tensor(xb[:], xbA[:], dinv_t[:, t_mine:t_mine + 1],
                                           bias_t[:, 0, :], AOT.mult, AOT.add,
                                           accum_out=r1[:])
            sq = ln.tile([128, feat], F32, tag="sq")
            r2 = ln.tile([128, 1], F32, tag="r2")
            nc.scalar.activation(sq[:], xb[:], AFT.Square, accum_out=r2[:])
            mu = ln.tile([128, 1], F32, tag="mu")
            nc.vector.tensor_scalar(mu[:], r1[:], 1.0 / feat, None, AOT.mult)
            musq = ln.tile([128, 1], F32, tag="musq")
            nc.vector.tensor_tensor(musq[:], mu[:], mu[:], AOT.mult)
            var = ln.tile([128, 1], F32, tag="var")
            nc.vector.tensor_scalar(var[:], r2[:], 1.0 / feat, musq[:],
                                    AOT.mult, AOT.subtract)
            st = ln.tile([128, 1], F32, tag="st")
            nc.scalar.activation(st[:], var[:], AFT.Sqrt, bias=eps_t[:])
            rstd = ln.tile([128, 1], F32, tag="rstd")
            nc.vector.reciprocal(rstd[:], st[:])
            xn = ln.tile([128, feat], F32, tag="xn")
            nc.vector.tensor_scalar(xn[:], xb[:], mu[:], rstd[:],
                                    AOT.subtract, AOT.mult)
            y = ln.tile([128, feat], F32, tag="y")
            nc.vector.tensor_tensor(y[:], xn[:], bias_t[:, 1, :], AOT.mult)
            nc.vector.tensor_tensor(y[:], y[:], bias_t[:, 2, :], AOT.add)
            if gelu:
                h = ln.tile([128, feat], F32, tag="h")
                nc.scalar.activation(h[:], y[:], AFT.Gelu)
                out_cb(lt, h)
            else:
                out_cb(lt, y)

    # ---- L1 -> transform to table2 rows (dinv * h1g @ W2) ----
    def l1_out(lt, h):
        t_mine = (NT - TPC) + lt
        h1T = work.tile([128, n_h_ch, 128], BF16, tag="h1T")
        for cc in range(n_h_ch):
            pst = ps128.tile([128, 128], F32, tag="psT")
            nc.tensor.transpose(pst[:], h[:, cc * 128:(cc + 1) * 128], ident[:])
            nc.vector.tensor_copy(h1T[:, cc, :], pst[:])
        ps2 = ps128.tile([128, HID], F32, tag="psC")
        for cc in range(n_h_ch):
            nc.tensor.matmul(ps2[:], h1T[:, cc, :], w2s[:, cc, :],
                             start=(cc == 0), stop=(cc == n_h_ch - 1))
        h2 = work.tile([128, HID], BF16, tag="h2")
        nc.scalar.activation(h2[:], ps2[:], AFT.Copy, scale=dinv_t[:, t_mine:t_mine + 1])
        nc.sync.dma_start(ag2_in[lt * 128:(lt + 1) * 128, :], h2[:])

    agg_layer(tab1c, HID2, geom["B1"], geom["S1"], geom["CB1"], geom["NC1"],
              dl1, io["idx1"], bias1, True, tab1c[3][MYBASE - 3 * CH:, :], l1_out)

    nc.gpsimd.collective_compute(
        "AllGather", AOT.bypass,
        replica_groups=[list(range(N_CORES))],
        ins=[ag2_in.opt()], outs=[ag2_out.opt()])

    # ---- L2 aggregation -> final output ----
    def l2_out(lt, y):
        o = work.tile([128, HID], F32, tag="o")
        nc.vector.tensor_copy(o[:], y[:])
        nc.sync.dma_start(io["out"][lt * 128:(lt + 1) * 128, :], o[:])

    agg_layer([ag2_out[cc * CH:(cc + 1) * CH, :] for cc in range(NCHUNK)],
              HID, geom["B2"], geom["S2"], geom["CB2"], geom["NC2"],
              dl2, io["idx2"], bias2, False, ag2_in[:], l2_out)
    ctx.close()


# ============================ top-level kernel ============================

def declare_io(nc, geom):
    io = {
        "xt": nc.dram_tensor("xt", [IN_DIM, PADN], BF16, kind="ExternalInput").ap(),
        "w1": nc.dram_tensor("w1", [IN_DIM, HID2], BF16, kind="ExternalInput").ap(),
        "w2": nc.dram_tensor("w2", [HID2, HID], BF16, kind="ExternalInput").ap(),
        "bias1": nc.dram_tensor("bias1", [128, 3, HID2], F32, kind="ExternalInput").ap(),
        "bias2": nc.dram_tensor("bias2", [128, 3, HID], F32, kind="ExternalInput").ap(),
        "iota_b": nc.dram_tensor("iota_b", [128, 128], BF16, kind="ExternalInput").ap(),
        "ident": nc.dram_tensor("ident", [128, 128], F32, kind="ExternalInput").ap(),
        "dinv_t": nc.dram_tensor("dinv_t", [128, NT], F32, kind="ExternalInput").ap(),
        "idx1": nc.dram_tensor("idx1", [128, geom["NB1"] * 8], dt.int16,
                               kind="ExternalInput").ap(),
        "dl1": nc.dram_tensor("dl1", [128, geom["NB1"]], BF16, kind="ExternalInput").ap(),
        "idx2": nc.dram_tensor("idx2", [128, geom["NB2"] * 8], dt.int16,
                               kind="ExternalInput").ap(),
        "dl2": nc.dram_tensor("dl2", [128, geom["NB2"]], BF16, kind="ExternalInput").ap(),
        "c2048": nc.dram_tensor("c2048", [1, 1], dt.int32, kind="ExternalInput").ap(),
        "out": nc.dram_tensor("out", [SHARD, HID], F32, kind="ExternalOutput").ap(),
    }
    return io


def kernel(x, edge_index, W1, b1, g1, be1, W2, b2, g2, be2,
           trace=False, _return_raw=False):
    bf = dt.np(BF16)
    x = np.asarray(x, np.float32)
    src = np.asarray(edge_index[0], np.int64)
    dst = np.asarray(edge_index[1], np.int64)
    N = x.shape[0]

    deg = (np.bincount(dst, minlength=N) + 1).astype(np.float32)
    dinv = (1.0 / np.sqrt(deg)).astype(np.float32)

    order = np.argsort(-deg, kind="stable")
    node_tile = np.empty(N, np.int32)
    node_slot = np.empty(N, np.int32)
    ar = np.arange(N, dtype=np.int64)
    node_tile[order] = (ar % NT).astype(np.int32)
    node_slot[order] = (ar // NT).astype(np.int32)
    core_of = node_tile % N_CORES
    lt_of = node_tile // N_CORES

    dinv_st = np.ones((TILE, NT), np.float32)
    dinv_st[node_slot, node_tile] = dinv
    row2 = core_of.astype(np.int64) * SHARD + lt_of.astype(np.int64) * TILE + node_slot

    # --- per-core packing ---
    cores = []
    cnts1, cnts2 = [], []
    for k in range(N_CORES):
        others = np.setdiff1d(np.arange(NT, dtype=np.int64),
                              np.arange(k, NT, N_CORES, dtype=np.int64),
                              assume_unique=True)
        mine = np.arange(k, NT, N_CORES, dtype=np.int64)
        tord = np.concatenate([others, mine])
        tpos = np.empty(NT, np.int64)
        tpos[tord] = np.arange(NT, dtype=np.int64)
        row1 = tpos[node_tile] * TILE + node_slot

        m = core_of[dst] == k
        elt = lt_of[dst[m]].astype(np.int64)
        eslot = node_slot[dst[m]].astype(np.float32)
        esrc = src[m]

        def sort_pack(srcrow):
            c = srcrow // CH
            i16 = (srcrow - c * CH).astype(np.int16)
            key = elt * NCHUNK + c
            o = np.argsort(key, kind="stable")
            cnts = np.bincount(key, minlength=TPC * NCHUNK).reshape(TPC, NCHUNK)
            return i16[o], eslot[o], cnts

        i16a, sla, ca = sort_pack(row1[esrc])
        i16b, slb, cb = sort_pack(row2[esrc])
        cnts1.append(ca)
        cnts2.append(cb)

        xs = np.zeros((PADN, IN_DIM), np.float32)
        xs[row1] = x
        cores.append(dict(
            xt=np.ascontiguousarray(xs.T).astype(bf),
            dinv_t=np.ascontiguousarray(dinv_st[:, tord]),
            e1=(i16a, sla, ca), e2=(i16b, slb, cb),
            nodes=np.nonzero(core_of == k)[0]))

    B1, S1, CB1, NC1, NB1 = finalize_geometry(cnts1)
    B2, S2, CB2, NC2, NB2 = finalize_geometry(cnts2)
    geom = dict(B1=B1, S1=S1, CB1=CB1, NC1=NC1, NB1=NB1,
                B2=B2, S2=S2, CB2=CB2, NC2=NC2, NB2=NB2)

    iota_np = np.tile(np.arange(128, dtype=np.float32)[None, :], (128, 1)).astype(bf)
    ident_np = np.eye(128, dtype=np.float32)
    bias1_np = np.broadcast_to(
        np.stack([np.asarray(b1, np.float32), np.asarray(g1, np.float32),
                  np.asarray(be1, np.float32)])[None], (128, 3, HID2)).copy()
    bias2_np = np.broadcast_to(
        np.stack([np.asarray(b2, np.float32), np.asarray(g2, np.float32),
                  np.asarray(be2, np.float32)])[None], (128, 3, HID)).copy()

    in_maps = []
    for k in range(N_CORES):
        pc = cores[k]
        idx1, dl1 = build_core_arrays(pc["e1"], B1, S1, CB1, NB1)
        idx2, dl2 = build_core_arrays(pc["e2"], B2, S2, CB2, NB2)
        in_maps.append({
            "xt": pc["xt"], "w1": np.asarray(W1, np.float32).astype(bf),
            "w2": np.asarray(W2, np.float32).astype(bf),
            "bias1": bias1_np, "bias2": bias2_np,
            "iota_b": iota_np, "ident": ident_np,
            "dinv_t": pc["dinv_t"],
            "idx1": idx1, "dl1": dl1, "idx2": idx2, "dl2": dl2,
            "c2048": np.array([[NI]], np.int32),
        })

    nc = bacc.Bacc("TRN2", debug=False, num_devices=N_CORES)
    io = declare_io(nc, geom)
    with tile.TileContext(nc) as tc:
        build_program(tc, io, geom)
    nc.compile()

    res = run_bass_kernel_spmd(nc, in_maps, core_ids=list(range(N_CORES)),
                               trace=trace)
    out = np.empty((N, HID), np.float32)
    for k in range(N_CORES):
        pc = cores[k]
        ok = np.asarray(res.results[k]["out"])
        pos = lt_of[pc["nodes"]] * TILE + node_slot[pc["nodes"]]
        out[pc["nodes"]] = ok[pos]
    if _return_raw:
        return out, res
    return out


def build_core_arrays(epack, B, S, CB, NB):
    bf = dt.np(BF16)
    i16, slot, cnts = epack
    idx_a = np.zeros((16, NB * 8), np.int16)
    dl_a = np.full((TILE, NB), PADSLOT, np.float32)
    starts = np.zeros(TPC * NCHUNK + 1, np.int64)
    np.cumsum(cnts.reshape(-1), out=starts[1:])
    for lt in range(TPC):
        for cc in range(NCHUNK):
            m = int(cnts[lt, cc])
            if m == 0:
                continue
            s0 = int(starts[lt * NCHUNK + cc])
            p = (int(CB[cc] + S[lt, cc])) * TILE + np.arange(m)
            idx_a[p % 16, p // 16] = i16[s0:s0 + m]
            dl_a[p % TILE, p // TILE] = slot[s0:s0 + m]
    return np.tile(idx_a, (8, 1)), dl_a.astype(bf)


def finalize_geometry(cnts_list):
    allc = np.stack(cnts_list)  # [8, TPC, NCHUNK]
    B = (-(-allc.max(axis=0) // TILE)).astype(np.int64)
    S = np.zeros((TPC, NCHUNK), np.int64)
    CB = np.zeros(NCHUNK + 1, np.int64)
    NC = np.zeros(NCHUNK, np.int64)
    for cc in range(NCHUNK):
        S[:, cc] = np.cumsum(B[:, cc]) - B[:, cc]
        nb = int(B[:, cc].sum())
        NC[cc] = -(-nb // CALLB)
        CB[cc + 1] = CB[cc] + NC[cc] * CALLB
    return B, S, CB, NC, int(CB[NCHUNK])


# revision 14
# speedup vs baseline: 1.3916x; 1.0080x over previous
"""Trainium2 Bass kernel for a 2-layer GCN encoder (GCNConv -> LN -> GELU -> GCNConv -> LN).

Strategy (8 NeuronCores, SPMD), v2:
  - Nodes assigned to 784 global tiles of 128 (degree-balanced); core k owns
    tiles t with t%8==k (98 tiles = 12544 dst rows per core).
  - Layer 1: every core computes the FULL transformed table H1*dinv locally
    (X@W1 on all 784 tiles; no collective). Per-core table row order is a
    per-core permutation with the core's own tiles LAST so "my rows" sit at a
    core-independent offset.
  - Aggregation: normalization folded into the table (rows pre-scaled by
    dinv[src]) and the output (post-scaled by dinv[dst]); selector matrices
    are pure one-hot (single is_equal). Self-loops handled by adding the
    node's own table row (sequential read), not as gather edges.
  - Gathers: bf16 rows via dma_gather in fixed 3072-idx multi-packet calls,
    fully padded (pad idx=0, pad slot=200 -> zero selector column), so no
    per-call count registers and no NaN-guard memsets.
  - Layer 2: transform locally (h1g @ W2, scaled by dinv), AllGather the
    bf16 table (core-major rows), aggregate the same way.
"""

from contextlib import ExitStack

import numpy as np

import concourse.bass as bass
import concourse.bacc as bacc
import concourse.mybir as mybir
import concourse.tile as tile
from concourse.bass_utils import run_bass_kernel_spmd

dt = mybir.dt
F32 = dt.float32
BF16 = dt.bfloat16

# -------- problem geometry (hardcoded for the graded problem) --------
N_FULL = 100000
IN_DIM = 256
HID2 = 256
HID = 128
N_CORES = 8
TILE = 128
NT = 784           # global tiles
TPC = 98           # tiles per core
SHARD = TPC * TILE # 12544
PADN = NT * TILE   # 100352
NCHUNK = 4
CH = PADN // NCHUNK  # 25088 (int16-safe)
CALLB = 24         # blocks per gather call (3072 idxs; multi-packet)
NI = CALLB * TILE  # 3072
MYBASE = (NT - TPC) * TILE  # 87808: per-core table rows of own tiles
PADSLOT = 200.0


# ============================ bass program builder ============================

def build_program(tc, io, geom):
    nc = tc.nc
    AOT = mybir.AluOpType
    AFT = mybir.ActivationFunctionType
    eps = 1e-5
    n_in_ch = IN_DIM // 128
    n_h_ch = HID2 // 128

    ctx = ExitStack()
    consts = ctx.enter_context(tc.tile_pool(name="consts", bufs=1))
    work = ctx.enter_context(tc.tile_pool(name="work", bufs=2))
    ln = ctx.enter_context(tc.tile_pool(name="ln", bufs=3))
    msgp = ctx.enter_context(tc.tile_pool(name="msgp", bufs=8))
    idxp = ctx.enter_context(tc.tile_pool(name="idxp", bufs=4))
    selp = ctx.enter_context(tc.tile_pool(name="selp", bufs=3))
    trowp = ctx.enter_context(tc.tile_pool(name="trowp", bufs=2))
    ps256 = ctx.enter_context(tc.tile_pool(name="ps256", bufs=2, space="PSUM"))
    ps128 = ctx.enter_context(tc.tile_pool(name="ps128", bufs=2, space="PSUM"))
    dram = ctx.enter_context(tc.tile_pool(name="dram", bufs=1, space="DRAM"))

    # ---- constants ----
    w1s = consts.tile([128, n_in_ch, HID2], BF16)
    nc.sync.dma_start(w1s[:], io["w1"].rearrange("(c p) n -> p c n", p=128))
    w2s = consts.tile([128, n_h_ch, HID], BF16)
    nc.sync.dma_start(w2s[:], io["w2"].rearrange("(c p) n -> p c n", p=128))
    bias1 = consts.tile([128, 3, HID2], F32)
    nc.sync.dma_start(bias1[:], io["bias1"])
    bias2 = consts.tile([128, 3, HID], F32)
    nc.sync.dma_start(bias2[:], io["bias2"])
    ident = consts.tile([128, 128], F32)
    nc.sync.dma_start(ident[:], io["ident"])
    iota_b = consts.tile([128, 128], BF16)
    nc.sync.dma_start(iota_b[:], io["iota_b"])
    dinv_t = consts.tile([128, NT], F32)
    nc.sync.dma_start(dinv_t[:], io["dinv_t"])
    dl1 = consts.tile([128, geom["NB1"]], BF16)
    nc.sync.dma_start(dl1[:], io["dl1"])
    dl2 = consts.tile([128, geom["NB2"]], BF16)
    nc.sync.dma_start(dl2[:], io["dl2"])
    eps_t = consts.tile([128, 1], F32)
    nc.vector.memset(eps_t[:], eps)
    c2048 = consts.tile([1, 1], dt.int32)
    nc.sync.dma_start(c2048[:], io["c2048"])
    r2048 = nc.alloc_register(mybir.EngineType.Pool, "gNI")
    nc.gpsimd.reg_load(r2048, c2048[:1, :1])

    # ---- DRAM buffers ----
    tab1c = [dram.tile([CH, HID2], BF16, name=f"tab1c{i}") for i in range(NCHUNK)]
    ag2_in = dram.tile([SHARD, HID], BF16)
    ag2_out = dram.tile([PADN, HID], BF16, addr_space="Shared")

    # ---- stage A: full local table1 = dinv * (X @ W1), bf16; 4 tiles/DMA ----
    for tb in range(0, NT, 14):
        xt_t = work.tile([128, n_in_ch, 14 * 128], BF16, tag="xt")
        nc.sync.dma_start(
            xt_t[:],
            io["xt"][:, tb * 128:(tb + 14) * 128].rearrange("(c p) n -> p c n", p=128))
        h1t = work.tile([128, 14, HID2], BF16, tag="h1t")
        for u in range(14):
            t = tb + u
            ps = ps256.tile([128, HID2], F32, tag="psA")
            for cc in range(n_in_ch):
                nc.tensor.matmul(ps[:], xt_t[:, cc, u * 128:(u + 1) * 128],
                                 w1s[:, cc, :],
                                 start=(cc == 0), stop=(cc == n_in_ch - 1))
            nc.vector.tensor_scalar(h1t[:, u, :], ps[:],
                                    dinv_t[:, t:t + 1], None, AOT.mult)
        tc_, tr = divmod(tb * 128, CH)
        nc.sync.dma_start(
            tab1c[tc_][tr:tr + 14 * 128, :].rearrange("(b p) f -> p b f", p=128),
            h1t[:])

    # ---- generic aggregation layer ----
    def agg_layer(tab_list, feat, B, S, CB, NC, dl_t, io_idx, bias_t, gelu, trow_src, out_cb):
        # emit all gather calls (Tile pipelines via pool WAR deps).
        # Warmup: two windows per chunk in chunk-completion order, so the Q7
        # gathers from early chunks while stage A still builds later chunks.
        msg_tiles = {}
        maxw = int(max(NC))
        WARM = 2
        emit_order = [(cc, w) for cc in range(NCHUNK)
                      for w in range(min(WARM, int(NC[cc])))]
        emit_order += [(cc, w) for w in range(WARM, maxw)
                       for cc in range(NCHUNK) if w < NC[cc]]
        for cc, w in emit_order:
            if True:
                it = idxp.tile([128, NI // 16], dt.int16, tag="idx")
                col0 = int(CB[cc] + w * CALLB) * 8
                nc.sync.dma_start(it[:], io_idx[:, col0:col0 + NI // 16])
                msg = msgp.tile([128, CALLB, feat], BF16, tag="msg")
                nc.gpsimd.dma_gather(
                    msg[:], tab_list[cc][:], it[:],
                    NI, r2048, feat, single_packet=False)
                msg_tiles[(cc, w)] = msg

        for lt in range(TPC):
            bt = int(B[lt].sum())
            assert bt > 0
            ps = ps256.tile([128, feat], F32, tag="psAgg")
            done = 0
            for cc in range(NCHUNK):
                bc = int(B[lt, cc])
                if bc == 0:
                    continue
                sel = selp.tile([128, bc, 128], BF16, tag="sel")
                g0 = int(CB[cc] + S[lt, cc])
                nc.vector.tensor_tensor(
                    sel[:],
                    iota_b[:].rearrange("p (b m) -> p b m", b=1).to_broadcast((128, bc, 128)),
                    dl_t[:, g0:g0 + bc].rearrange("p (b m) -> p b m", m=1).to_broadcast((128, bc, 128)),
                    AOT.is_equal)
                for bi in range(bc):
                    w, j = divmod(g0 + bi, CALLB)
                    w -= int(CB[cc]) // CALLB
                    msg = msg_tiles[(cc, w)]
                    nc.tensor.matmul(ps[:], sel[:, bi, :], msg[:, j, :],
                                     start=(done == 0), stop=(done == bt - 1))
                    done += 1
            # + own row (self loop), scale by dinv[dst], +bias, LN (+gelu)
            trow = trowp.tile([128, feat], BF16, tag="trow")
            nc.sync.dma_start(trow[:], trow_src[lt * 128:(lt + 1) * 128, :])
            t_mine = (NT - TPC) + lt  # position of my lt-th tile in per-core order
            xbA = ln.tile([128, feat], F32, tag="xbA")
            nc.vector.tensor_tensor(xbA[:], ps[:], trow[:], AOT.add)
            xb = ln.tile([128, feat], F32, tag="xb")
            r1 = ln.tile([128, 1], F32, tag="r1")
            nc.vector.scalar_tensor_tensor(xb[:], xbA[:], dinv_t[:, t_mine:t_mine + 1],
                                           bias_t[:, 0, :], AOT.mult, AOT.add,
                                           accum_out=r1[:])
            sq = ln.tile([128, feat], F32, tag="sq")
            r2 = ln.tile([128, 1], F32, tag="r2")
            nc.scalar.activation(sq[:], xb[:], AFT.Square, accum_out=r2[:])
            mu = ln.tile([128, 1], F32, tag="mu")
            nc.vector.tensor_scalar(mu[:], r1[:], 1.0 / feat, None, AOT.mult)
            musq = ln.tile([128, 1], F32, tag="musq")
            nc.vector.tensor_tensor(musq[:], mu[:], mu[:], AOT.mult)
            var = ln.tile([128, 1], F32, tag="var")
            nc.vector.tensor_scalar(var[:], r2[:], 1.0 / feat, musq[:],
                                    AOT.mult, AOT.subtract)
            st = ln.tile([128, 1], F32, tag="st")
            nc.scalar.activation(st[:], var[:], AFT.Sqrt, bias=eps_t[:])
            rstd = ln.tile([128, 1], F32, tag="rstd")
            nc.vector.reciprocal(rstd[:], st[:])
            xn = ln.tile([128, feat], F32, tag="xn")
            nc.vector.tensor_scalar(xn[:], xb[:], mu[:], rstd[:],
                                    AOT.subtract, AOT.mult)
            y = ln.tile([128, feat], F32, tag="y")
            nc.vector.tensor_tensor(y[:], xn[:], bias_t[:, 1, :], AOT.mult)
            nc.vector.tensor_tensor(y[:], y[:], bias_t[:, 2, :], AOT.add)
            if gelu:
                h = ln.tile([128, feat], F32, tag="h")
                nc.scalar.activation(h[:], y[:], AFT.Gelu)
                out_cb(lt, h)
            else:
                out_cb(lt, y)

    # ---- L1 -> transform to table2 rows (dinv * h1g @ W2) ----
    def l1_out(lt, h):
        t_mine = (NT - TPC) + lt
        h1T = work.tile([128, n_h_ch, 128], BF16, tag="h1T")
        for cc in range(n_h_ch):
            pst = ps128.tile([128, 128], F32, tag="psT")
            nc.tensor.transpose(pst[:], h[:, cc * 128:(cc + 1) * 128], ident[:])
            nc.vector.tensor_copy(h1T[:, cc, :], pst[:])
        ps2 = ps128.tile([128, HID], F32, tag="psC")
        for cc in range(n_h_ch):
            nc.tensor.matmul(ps2[:], h1T[:, cc, :], w2s[:, cc, :],
                             start=(cc == 0), stop=(cc == n_h_ch - 1))
        h2 = work.tile([128, HID], BF16, tag="h2")
        nc.scalar.activation(h2[:], ps2[:], AFT.Copy, scale=dinv_t[:, t_mine:t_mine + 1])
        nc.sync.dma_start(ag2_in[lt * 128:(lt + 1) * 128, :], h2[:])

    agg_layer(tab1c, HID2, geom["B1"], geom["S1"], geom["CB1"], geom["NC1"],
              dl1, io["idx1"], bias1, True, tab1c[3][MYBASE - 3 * CH:, :], l1_out)

    nc.gpsimd.collective_compute(
        "AllGather", AOT.bypass,
        replica_groups=[list(range(N_CORES))],
        ins=[ag2_in.opt()], outs=[ag2_out.opt()])

    # ---- L2 aggregation -> final output ----
    def l2_out(lt, y):
        o = work.tile([128, HID], F32, tag="o")
        nc.vector.tensor_copy(o[:], y[:])
        nc.sync.dma_start(io["out"][lt * 128:(lt + 1) * 128, :], o[:])

    agg_layer([ag2_out[cc * CH:(cc + 1) * CH, :] for cc in range(NCHUNK)],
              HID, geom["B2"], geom["S2"], geom["CB2"], geom["NC2"],
              dl2, io["idx2"], bias2, False, ag2_in[:], l2_out)
    ctx.close()


# ============================ top-level kernel ============================

def declare_io(nc, geom):
    io = {
        "xt": nc.dram_tensor("xt", [IN_DIM, PADN], BF16, kind="ExternalInput").ap(),
        "w1": nc.dram_tensor("w1", [IN_DIM, HID2], BF16, kind="ExternalInput").ap(),
        "w2": nc.dram_tensor("w2", [HID2, HID], BF16, kind="ExternalInput").ap(),
        "bias1": nc.dram_tensor("bias1", [128, 3, HID2], F32, kind="ExternalInput").ap(),
        "bias2": nc.dram_tensor("bias2", [128, 3, HID], F32, kind="ExternalInput").ap(),
        "iota_b": nc.dram_tensor("iota_b", [128, 128], BF16, kind="ExternalInput").ap(),
        "ident": nc.dram_tensor("ident", [128, 128], F32, kind="ExternalInput").ap(),
        "dinv_t": nc.dram_tensor("dinv_t", [128, NT], F32, kind="ExternalInput").ap(),
        "idx1": nc.dram_tensor("idx1", [128, geom["NB1"] * 8], dt.int16,
                               kind="ExternalInput").ap(),
        "dl1": nc.dram_tensor("dl1", [128, geom["NB1"]], BF16, kind="ExternalInput").ap(),
        "idx2": nc.dram_tensor("idx2", [128, geom["NB2"] * 8], dt.int16,
                               kind="ExternalInput").ap(),
        "dl2": nc.dram_tensor("dl2", [128, geom["NB2"]], BF16, kind="ExternalInput").ap(),
        "c2048": nc.dram_tensor("c2048", [1, 1], dt.int32, kind="ExternalInput").ap(),
        "out": nc.dram_tensor("out", [SHARD, HID], F32, kind="ExternalOutput").ap(),
    }
    return io


def kernel(x, edge_index, W1, b1, g1, be1, W2, b2, g2, be2,
           trace=False, _return_raw=False):
    bf = dt.np(BF16)
    x = np.asarray(x, np.float32)
    src = np.asarray(edge_index[0], np.int64)
    dst = np.asarray(edge_index[1], np.int64)
    N = x.shape[0]

    deg = (np.bincount(dst, minlength=N) + 1).astype(np.float32)
    dinv = (1.0 / np.sqrt(deg)).astype(np.float32)

    order = np.argsort(-deg, kind="stable")
    node_tile = np.empty(N, np.int32)
    node_slot = np.empty(N, np.int32)
    ar = np.arange(N, dtype=np.int64)
    node_tile[order] = (ar % NT).astype(np.int32)
    node_slot[order] = (ar // NT).astype(np.int32)
    core_of = node_tile % N_CORES
    lt_of = node_tile // N_CORES

    dinv_st = np.ones((TILE, NT), np.float32)
    dinv_st[node_slot, node_tile] = dinv
    row2 = core_of.astype(np.int64) * SHARD + lt_of.astype(np.int64) * TILE + node_slot

    # --- per-core packing ---
    cores = []
    cnts1, cnts2 = [], []
    for k in range(N_CORES):
        others = np.setdiff1d(np.arange(NT, dtype=np.int64),
                              np.arange(k, NT, N_CORES, dtype=np.int64),
                              assume_unique=True)
        mine = np.arange(k, NT, N_CORES, dtype=np.int64)
        tord = np.concatenate([others, mine])
        tpos = np.empty(NT, np.int64)
        tpos[tord] = np.arange(NT, dtype=np.int64)
        row1 = tpos[node_tile] * TILE + node_slot

        m = core_of[dst] == k
        elt = lt_of[dst[m]].astype(np.int64)
        eslot = node_slot[dst[m]].astype(np.float32)
        esrc = src[m]

        def sort_pack(srcrow):
            c = srcrow // CH
            i16 = (srcrow - c * CH).astype(np.int16)
            key = elt * NCHUNK + c
            o = np.argsort(key, kind="stable")
            cnts = np.bincount(key, minlength=TPC * NCHUNK).reshape(TPC, NCHUNK)
            return i16[o], eslot[o], cnts

        i16a, sla, ca = sort_pack(row1[esrc])
        i16b, slb, cb = sort_pack(row2[esrc])
        cnts1.append(ca)
        cnts2.append(cb)

        xs = np.zeros((PADN, IN_DIM), np.float32)
        xs[row1] = x
        cores.append(dict(
            xt=np.ascontiguousarray(xs.T).astype(bf),
            dinv_t=np.ascontiguousarray(dinv_st[:, tord]),
            e1=(i16a, sla, ca), e2=(i16b, slb, cb),
            nodes=np.nonzero(core_of == k)[0]))

    B1, S1, CB1, NC1, NB1 = finalize_geometry(cnts1)
    B2, S2, CB2, NC2, NB2 = finalize_geometry(cnts2)
    geom = dict(B1=B1, S1=S1, CB1=CB1, NC1=NC1, NB1=NB1,
                B2=B2, S2=S2, CB2=CB2, NC2=NC2, NB2=NB2)

    iota_np = np.tile(np.arange(128, dtype=np.float32)[None, :], (128, 1)).astype(bf)
    ident_np = np.eye(128, dtype=np.float32)
    bias1_np = np.broadcast_to(
        np.stack([np.asarray(b1, np.float32), np.asarray(g1, np.float32),
                  np.asarray(be1, np.float32)])[None], (128, 3, HID2)).copy()
    bias2_np = np.broadcast_to(
        np.stack([np.asarray(b2, np.float32), np.asarray(g2, np.float32),
                  np.asarray(be2, np.float32)])[None], (128, 3, HID)).copy()

    in_maps = []
    for k in range(N_CORES):
        pc = cores[k]
        idx1, dl1 = build_core_arrays(pc["e1"], B1, S1, CB1, NB1)
        idx2, dl2 = build_core_arrays(pc["e2"], B2, S2, CB2, NB2)
        in_maps.append({
            "xt": pc["xt"], "w1": np.asarray(W1, np.float32).astype(bf),
            "w2": np.asarray(W2, np.float32).astype(bf),
            "bias1": bias1_np, "bias2": bias2_np,
            "iota_b": iota_np, "ident": ident_np,
            "dinv_t": pc["dinv_t"],
            "idx1": idx1, "dl1": dl1, "idx2": idx2, "dl2": dl2,
            "c2048": np.array([[NI]], np.int32),
        })

    nc = bacc.Bacc("TRN2", debug=False, num_devices=N_CORES)
    io = declare_io(nc, geom)
    with tile.TileContext(nc) as tc:
        build_program(tc, io, geom)
    nc.compile()

    res = run_bass_kernel_spmd(nc, in_maps, core_ids=list(range(N_CORES)),
                               trace=trace)
    out = np.empty((N, HID), np.float32)
    for k in range(N_CORES):
        pc = cores[k]
        ok = np.asarray(res.results[k]["out"])
        pos = lt_of[pc["nodes"]] * TILE + node_slot[pc["nodes"]]
        out[pc["nodes"]] = ok[pos]
    if _return_raw:
        return out, res
    return out


def build_core_arrays(epack, B, S, CB, NB):
    bf = dt.np(BF16)
    i16, slot, cnts = epack
    idx_a = np.zeros((16, NB * 8), np.int16)
    dl_a = np.full((TILE, NB), PADSLOT, np.float32)
    starts = np.zeros(TPC * NCHUNK + 1, np.int64)
    np.cumsum(cnts.reshape(-1), out=starts[1:])
    for lt in range(TPC):
        for cc in range(NCHUNK):
            m = int(cnts[lt, cc])
            if m == 0:
                continue
            s0 = int(starts[lt * NCHUNK + cc])
            p = (int(CB[cc] + S[lt, cc])) * TILE + np.arange(m)
            idx_a[p % 16, p // 16] = i16[s0:s0 + m]
            dl_a[p % TILE, p // TILE] = slot[s0:s0 + m]
    return np.tile(idx_a, (8, 1)), dl_a.astype(bf)


def finalize_geometry(cnts_list):
    allc = np.stack(cnts_list)  # [8, TPC, NCHUNK]
    B = (-(-allc.max(axis=0) // TILE)).astype(np.int64)
    S = np.zeros((TPC, NCHUNK), np.int64)
    CB = np.zeros(NCHUNK + 1, np.int64)
    NC = np.zeros(NCHUNK, np.int64)
    for cc in range(NCHUNK):
        S[:, cc] = np.cumsum(B[:, cc]) - B[:, cc]
        nb = int(B[:, cc].sum())
        NC[cc] = -(-nb // CALLB)
        CB[cc + 1] = CB[cc] + NC[cc] * CALLB
    return B, S, CB, NC, int(CB[NCHUNK])
